# revision 7
# baseline (speedup 1.0000x reference)
"""DeepSetPred Trainium2 kernel: token encoder MLP + segment-sum + predictor
MLP on 8 NeuronCores, zero collectives.

Sharding: the host cuts the (sorted-by-segment) token axis at segment
boundaries, so every segment belongs to exactly one core. Each shard is
padded to a common length with tokens whose one-hot selector row is all
zero. Each core computes the complete segment sums for its own contiguous
range of <=SLOTS segments, runs the predictor on those rows, and writes its
private slice of the output; the host concatenates.

Structure: the encoder's third linear layer commutes with the segment sum
(it sits after the last tanh), so
    segsum(h2 @ W3 + b3) == segsum(h2) @ W3 + counts * b3
and W3 further folds into the predictor's first layer:
    enc @ P1 + pb1 == segsum(h2) @ (W3 @ P1) + counts * (b3 @ P1) + pb1.
The per-token path is only L1 + L2 + a one-hot segsum matmul over h2
(14336 PE rows per 512-token chunk). L2 is computed token-major (h1 tile
stationary, W2 moving) so the segsum needs no transpose; its bias is added
by the DVE from a broadcast tile (ACT bias is per-partition only), then ACT
applies tanh. The PE stream is skewed L1(i) | L2(i-2) | seg(i-3) so neither
the w2 weight DMA at startup nor the DVE+ACT hop ever stalls the PE. All
weights are host-pre-shuffled into dense [128, X] partition-contiguous
blocks; w1/w2 are split across the scalar+vector DMA queues to parallelize
the startup load, and xt uses 2KB partition lines.
"""

import numpy as np

import concourse.mybir as mybir
import concourse.tile as tile
from concourse import bacc
from concourse import bass_utils
from concourse.masks import make_identity

# Problem shapes (hardcoded per contract).
T, E, H, C, O = 131072, 256, 512, 256, 32
S = 128            # num segments
N_CORES = 8
TOK = 512          # tokens per chunk
MIN_SLOTS = 32     # baseline segments-per-core capacity
SG = 4             # chunks per sel DMA group
F32 = mybir.dt.float32
F32R = mybir.dt.float32r
F16 = mybir.dt.float16

EC = E // 128   # 2
HC = H // 128   # 4
TT = TOK // 128  # 4 token sub-tiles per chunk

_CACHE = {}


def _mm(nc, out, lhsT, rhs, start, stop, skip=True):
    nc.tensor.matmul(out, lhsT, rhs,
                     start=start, stop=stop, skip_group_check=skip)


def _build_nc(t_sh, SLOTS):
    assert t_sh % 128 == 0
    n_full = t_sh // TOK
    tail = t_sh - n_full * TOK
    chunks = [(i * TOK, TOK) for i in range(n_full)]
    if tail:
        chunks.append((n_full * TOK, tail))
    NCH = len(chunks)
    NSG = (NCH + SG - 1) // SG

    nc = bacc.Bacc("TRN2", target_bir_lowering=False, debug=False,
                   num_devices=N_CORES)

    # xt packed: [128, NCH, EC, TOK] -> 2KB contiguous per partition/chunk
    xt_d = nc.dram_tensor("xt", [128, NCH * EC * TOK], F16,
                          kind="ExternalInput")
    # sel packed per chunk: [128, NCH, TT, SLOTS] flattened on the free dim
    sel_d = nc.dram_tensor("sel", [128, NCH * TT * SLOTS], F16,
                           kind="ExternalInput")
    cnt_d = nc.dram_tensor("cnt", [1, SLOTS], F32, kind="ExternalInput")
    # dense pre-shuffled weights: [128, ...] partition-major blocks
    w1_d = nc.dram_tensor("w1", [128, HC * EC * 128], F16,
                          kind="ExternalInput")      # h-major tiles
    w2_d = nc.dram_tensor("w2", [128, HC * H], F16, kind="ExternalInput")
    b1_d = nc.dram_tensor("b1", [128, HC], F32, kind="ExternalInput")
    b2f_d = nc.dram_tensor("b2f", [128, H], F32, kind="ExternalInput")
    wp1_d = nc.dram_tensor("wp1", [128, HC * HC * 128], F32,
                           kind="ExternalInput")   # W3 @ P1, k-major tiles
    b3p1_d = nc.dram_tensor("b3p1", [1, H], F32, kind="ExternalInput")
    p2_d = nc.dram_tensor("p2", [128, HC * HC * 128], F32,
                          kind="ExternalInput")
    p3_d = nc.dram_tensor("p3", [128, HC * O], F32, kind="ExternalInput")
    pb1_d = nc.dram_tensor("pb1", [128, HC], F32, kind="ExternalInput")
    pb2_d = nc.dram_tensor("pb2", [128, HC], F32, kind="ExternalInput")
    pb3_d = nc.dram_tensor("pb3", [1, O], F32, kind="ExternalInput")
    out_d = nc.dram_tensor("pred", [SLOTS, O], F32, kind="ExternalOutput")

    with tile.TileContext(nc) as tc:
        with tc.tile_pool(name="wts", bufs=1) as wp, \
             tc.tile_pool(name="xt", bufs=4) as xtp, \
             tc.tile_pool(name="sel", bufs=3) as selp, \
             tc.tile_pool(name="act", bufs=3) as actp, \
             tc.tile_pool(name="small", bufs=1) as smp, \
             tc.tile_pool(name="ps", bufs=2, space="PSUM") as psp, \
             tc.tile_pool(name="psacc", bufs=1, space="PSUM") as psa:

            # warm the ACT tanh table before the queues fill
            warm_sb = smp.tile([1, 1], F32, tag="warm", name="warm")
            nc.gpsimd.memset(warm_sb[:], 0.0)
            warm_o = smp.tile([1, 1], F32, tag="warmo", name="warmo")
            nc.scalar.activation(warm_o[:], warm_sb[:],
                                 mybir.ActivationFunctionType.Tanh)

            # ---- resident weights; every DMA is partition-contiguous.
            # w1/w2 split across the scalar+vector queues so both halves
            # land in parallel while the sync queue streams xt. ----
            # w1 (whole, 2KB lines) + b1 on the scalar ring; w2 (whole,
            # 4KB lines) leads the gpsimd ring so L2(0) is never blocked.
            w1_t = wp.tile([128, HC, EC, 128], F16, tag="w1", name="w1t")
            w1_r = w1_d.ap().rearrange("p (h e q) -> p h e q", h=HC, e=EC)
            nc.scalar.dma_start(w1_t[:], w1_r)
            b1_sb = smp.tile([128, HC], F32, tag="b1", name="b1")
            nc.scalar.dma_start(b1_sb[:], b1_d.ap())
            w2_t = wp.tile([128, HC, H], F16, tag="w2", name="w2t")
            w2_r = w2_d.ap().rearrange("p (k j) -> p k j", k=HC)
            nc.gpsimd.dma_start(w2_t[:], w2_r)
            b2f_sb = smp.tile([128, H], F32, tag="b2f", name="b2f")
            nc.gpsimd.dma_start(b2f_sb[:], b2f_d.ap())
            wp1_t = wp.tile([128, HC, HC, 128], F32R, tag="wp1", name="wp1t")
            nc.gpsimd.dma_start(
                wp1_t[:], wp1_d.ap().rearrange("p (k h q) -> p k h q",
                                               k=HC, h=HC))
            p2_t = wp.tile([128, HC, HC, 128], F32R, tag="p2", name="p2t")
            nc.gpsimd.dma_start(
                p2_t[:], p2_d.ap().rearrange("p (k h q) -> p k h q",
                                             k=HC, h=HC))
            p3_t = wp.tile([128, HC, O], F32R, tag="p3", name="p3t")
            nc.gpsimd.dma_start(
                p3_t[:], p3_d.ap().rearrange("p (k o) -> p k o", k=HC))
            b3p1row = smp.tile([1, H], F32, tag="b3p1", name="b3p1")
            nc.gpsimd.dma_start(b3p1row[:], b3p1_d.ap())
            pb1_sb = smp.tile([128, HC], F32, tag="pb1", name="pb1")
            nc.gpsimd.dma_start(pb1_sb[:], pb1_d.ap())
            pb2_sb = smp.tile([128, HC], F32, tag="pb2", name="pb2")
            nc.gpsimd.dma_start(pb2_sb[:], pb2_d.ap())
            pb3row = smp.tile([1, O], F32, tag="pb3row", name="pb3row")
            nc.gpsimd.dma_start(pb3row[:], pb3_d.ap())
            cntrow = smp.tile([1, SLOTS], F32, tag="cntrow", name="cntrow")
            nc.gpsimd.dma_start(cntrow[:], cnt_d.ap())
            ones1 = smp.tile([1, SLOTS], F32, tag="ones1", name="ones1")
            nc.gpsimd.memset(ones1[:], 1.0)
            ident = smp.tile([SLOTS, SLOTS], F32, tag="ident", name="ident")
            make_identity(nc, ident[:])

            # ---- persistent segment-sum accumulator Z[slot, h] ----
            enc_ps = psa.tile([SLOTS, H], F32, tag="encacc", name="encacc")

            xt_r = xt_d.ap().rearrange("p (c e t) -> p c e t", c=NCH, e=EC)
            sel_r = sel_d.ap().rearrange("p (c q s) -> p c q s",
                                         c=NCH, q=TT)

            sel_tiles = {}

            def dma_xt(ci):
                tok = chunks[ci][1]
                xt_t = xtp.tile([128, EC, tok], F16, tag="xt", name="xt",
                                padded_shape=[128, EC, TOK])
                nc.sync.dma_start(xt_t[:], xt_r[:, ci, :, 0:tok])
                return xt_t

            def dma_selg(g):
                lo = g * SG
                gsz = min(SG, NCH - lo)
                selg = selp.tile([128, gsz, TT, SLOTS], F16, tag="sel",
                                 name="sel", padded_shape=[128, SG, TT,
                                                           SLOTS])
                nc.sync.dma_start(selg[:], sel_r[:, lo:lo + gsz, :, :])
                sel_tiles[g] = selg

            def l1(xt_t, tok):
                h1_t = actp.tile([128, HC, tok], F16, tag="h1", name="h1",
                                 bufs=5, padded_shape=[128, HC, TOK])
                for h in range(HC):
                    ps1 = psp.tile([128, tok], F32, tag="l1", name="l1",
                                   bufs=3, padded_shape=[128, TOK])
                    for e in range(EC):
                        _mm(nc, ps1[:], w1_t[:, h, e, :], xt_t[:, e, :],
                            start=(e == 0), stop=(e == EC - 1))
                    nc.scalar.activation(h1_t[:, h, :], ps1[:],
                                         mybir.ActivationFunctionType.Tanh,
                                         bias=b1_sb[:, h:h + 1])
                return h1_t

            def l2(h1_t, tok):
                tt = tok // 128
                h2_t = actp.tile([128, tt, H], F16, tag="h2", name="h2",
                                 padded_shape=[128, TT, H])
                for t in range(tt):
                    ps2 = psp.tile([128, H], F32, tag="l2", name="l2",
                                   bufs=3)
                    for k in range(HC):
                        _mm(nc, ps2[:], h1_t[:, k, t * 128:(t + 1) * 128],
                            w2_t[:, k, :], start=(k == 0),
                            stop=(k == HC - 1))
                    g2 = actp.tile([128, H], F16, tag="g2", name="g2")
                    nc.vector.tensor_add(g2[:], ps2[:], b2f_sb[:])
                    nc.scalar.activation(h2_t[:, t, :], g2[:],
                                         mybir.ActivationFunctionType.Tanh)
                return h2_t

            def seg(ci, h2_t, tok, is_first, is_last):
                tt = tok // 128
                selg = sel_tiles[ci // SG]
                for t in range(tt):
                    _mm(nc, enc_ps[:], selg[:, ci % SG, t, :],
                        h2_t[:, t, :],
                        start=(is_first and t == 0),
                        stop=(is_last and t == tt - 1))

            # ---- main loop: PE stream L1(i) | L2(i-3) | seg(i-4) ----
            assert NCH >= 5
            xt_q = [dma_xt(0), dma_xt(1)]
            dma_selg(0)
            h1_q = []
            h2_q = []
            for ci in range(NCH):
                if ci + 2 < NCH:
                    xt_q.append(dma_xt(ci + 2))
                    if (ci + 2) % SG == 0:
                        dma_selg((ci + 2) // SG)
                h1_q.append((l1(xt_q[ci], chunks[ci][1]), chunks[ci][1]))
                if ci >= 3:
                    h1_t, tok1 = h1_q[ci - 3]
                    h2_q.append((l2(h1_t, tok1), tok1))
                if ci >= 4:
                    h2_t, tok2 = h2_q[ci - 4]
                    seg(ci - 4, h2_t, tok2,
                        is_first=(ci == 4), is_last=False)
            # epilogue: remaining L2/seg in dependency-friendly order
            for ci in range(NCH - 3, NCH):
                h2_q.append((l2(h1_q[ci][0], h1_q[ci][1]), h1_q[ci][1]))
                sc = ci - 1
                seg(sc, h2_q[sc][0], h2_q[sc][1],
                    is_first=False, is_last=False)
            seg(NCH - 1, h2_q[NCH - 1][0], h2_q[NCH - 1][1],
                is_first=False, is_last=True)

            # ---- predictor on this core's own <=SLOTS segment rows ----
            # Z = segsum(h2) [SLOTS, H]; q1 = tanh(Z @ WP1 + cnt*b3p1 + pb1)
            # slice-pipelined: copy k-slice, transpose it while the next
            # slice copies, so the chain latency overlaps
            z_sb = smp.tile([SLOTS, H], F32, tag="zsb", name="zsb")
            zT = smp.tile([128, HC, SLOTS], F32R, tag="zT", name="zT")
            nc.scalar.copy(z_sb[:], enc_ps[:])
            for k in range(HC):
                pst = psp.tile([128, SLOTS], F32, tag="l1", name="pst",
                               bufs=3)
                nc.tensor.transpose(pst[:], z_sb[:, k * 128:(k + 1) * 128],
                                    ident[:])
                nc.vector.tensor_copy(zT[:, k, :], pst[:])

            q1_sb = smp.tile([128, HC, SLOTS], F32R, tag="q1", name="q1")
            for h in range(HC):
                pp1 = psp.tile([128, SLOTS], F32, tag="l1", name="pp1",
                               bufs=3)
                nc.tensor.matmul(pp1[:], b3p1row[:, h * 128:(h + 1) * 128],
                                 cntrow[:], start=True, stop=False,
                                 skip_group_check=True)
                for k in range(HC):
                    _mm(nc, pp1[:], wp1_t[:, k, h, :], zT[:, k, :],
                        start=False, stop=(k == HC - 1))
                nc.scalar.activation(q1_sb[:, h, :], pp1[:],
                                     mybir.ActivationFunctionType.Tanh,
                                     bias=pb1_sb[:, h:h + 1])
            q2_sb = smp.tile([128, HC, SLOTS], F32R, tag="q2", name="q2")
            for h in range(HC):
                pp2 = psp.tile([128, SLOTS], F32, tag="l1", name="pp2",
                               bufs=3)
                for k in range(HC):
                    _mm(nc, pp2[:], p2_t[:, k, h, :], q1_sb[:, k, :],
                        start=(k == 0), stop=(k == HC - 1))
                nc.scalar.activation(q2_sb[:, h, :], pp2[:],
                                     mybir.ActivationFunctionType.Tanh,
                                     bias=pb2_sb[:, h:h + 1])

            # final: pred[slot, o] = q2.T @ P3 + pb3
            ppo = psp.tile([SLOTS, O], F32, tag="l2", name="ppo", bufs=3)
            nc.tensor.matmul(ppo[:], ones1[:], pb3row[:],
                             start=True, stop=False, skip_group_check=True)
            for k in range(HC):
                _mm(nc, ppo[:], q2_sb[:, k, :], p3_t[:, k, :],
                    start=False, stop=(k == HC - 1))
            pred_sb = smp.tile([SLOTS, O], F32, tag="pred", name="predsb")
            nc.vector.tensor_copy(pred_sb[:], ppo[:])
            nc.sync.dma_start(out_d.ap(), pred_sb[:])

    nc.compile()
    return nc


def kernel(words, seg_ids, W1, b1, W2, b2, W3, b3,
           P1, pb1, P2, pb2, P3, pb3, batch_size, alpha_iter, **_):
    words = np.asarray(words, dtype=np.float32)
    seg_ids = np.asarray(seg_ids).astype(np.int64)
    assert words.shape == (T, E), words.shape
    bs, ai = int(batch_size), int(alpha_iter)

    # --- host-side index prep: cut the sorted token axis at segment
    # boundaries so each core owns whole segments ---
    counts = np.bincount(seg_ids, minlength=S)[:S]
    starts = np.concatenate([[0], np.cumsum(counts)])   # [S+1]
    cuts = [0]
    for c in range(1, N_CORES):
        tgt = c * T // N_CORES
        j = int(np.searchsorted(starts, tgt, side="left"))
        if j > 0 and tgt - starts[j - 1] < starts[j] - tgt:
            j -= 1
        cuts.append(int(starts[j]))
    cuts.append(T)
    lens = np.diff(cuts)
    t_sh = int(np.ceil(lens.max() / 128) * 128)

    # contiguous segment range owned by each core
    seg_lo = [0] * N_CORES
    for c in range(N_CORES - 1, 0, -1):
        if lens[c] > 0:
            seg_lo[c] = int(seg_ids[cuts[c]])
        else:
            seg_lo[c] = S if c == N_CORES - 1 else seg_lo[c + 1]
    seg_hi = seg_lo[1:] + [S]
    slots_needed = max(seg_hi[c] - seg_lo[c] for c in range(N_CORES))
    SLOTS = min(128, max(MIN_SLOTS, ((slots_needed + 31) // 32) * 32))
    assert slots_needed <= SLOTS, (seg_lo, seg_hi)
    assert bs * ai == S

    xt = np.ascontiguousarray(words.T.astype(np.float16))    # [E, T] fp16
    n_full = t_sh // TOK
    tail = t_sh - n_full * TOK
    NCH = n_full + (1 if tail else 0)

    key = ("nc", t_sh, SLOTS)
    if key not in _CACHE:
        _CACHE[key] = _build_nc(t_sh, SLOTS)
    nc = _CACHE[key]

    # --- host-side weight pre-shuffles (dense [128, X] blocks) ---
    W1f, W2f = np.float32(W1), np.float32(W2)
    WP1 = (np.float32(W3) @ np.float32(P1)).astype(np.float32)  # [H, H]
    b3p1 = (np.float32(b3) @ np.float32(P1)).astype(np.float32)  # [H]

    def kmaj_tiles(Wm, dtype):
        # W [K, N] -> [128, K//128, N//128, 128] p-major
        K, N = Wm.shape
        return np.ascontiguousarray(
            Wm.reshape(K // 128, 128, N // 128, 128)
              .transpose(1, 0, 2, 3).reshape(128, -1).astype(dtype))

    common = {
        # w1 h-major: [128, HC, EC, 128]
        "w1": np.ascontiguousarray(
            W1f.astype(np.float16).reshape(EC, 128, HC, 128)
               .transpose(1, 2, 0, 3).reshape(128, -1)),
        "w2": np.ascontiguousarray(
            W2f.astype(np.float16).reshape(HC, 128, H)
               .transpose(1, 0, 2).reshape(128, -1)),
        "b1": np.ascontiguousarray(
            np.float32(b1).reshape(HC, 128).T),
        "b2f": np.ascontiguousarray(
            np.broadcast_to(np.float32(b2), (128, H))),
        "wp1": kmaj_tiles(WP1, np.float32),
        "b3p1": b3p1.reshape(1, H),
        "p2": kmaj_tiles(np.float32(P2), np.float32),
        "p3": np.ascontiguousarray(
            np.float32(P3).reshape(HC, 128, O)
              .transpose(1, 0, 2).reshape(128, -1)),
        "pb1": np.ascontiguousarray(np.float32(pb1).reshape(HC, 128).T),
        "pb2": np.ascontiguousarray(np.float32(pb2).reshape(HC, 128).T),
        "pb3": np.float32(pb3).reshape(1, O),
    }
    in_maps = []
    for c in range(N_CORES):
        lo, hi = cuts[c], cuts[c + 1]
        n = hi - lo
        # xt packed [128, NCH, EC, TOK]: 2KB lines per partition/chunk
        xt_flat = np.zeros((E, NCH * TOK), dtype=np.float16)
        xt_flat[:, :n] = xt[:, lo:hi]
        xt_c = np.ascontiguousarray(
            xt_flat.reshape(EC, 128, NCH, TOK)
                   .transpose(1, 2, 0, 3).reshape(128, -1))
        # packed one-hot selector: sel8[p, ci, q, s]
        sel_flat = np.zeros((NCH * TOK, SLOTS), dtype=np.float16)
        sel_flat[:n, :] = (seg_ids[lo:hi, None] ==
                           (seg_lo[c] + np.arange(SLOTS))[None, :])
        sel_c = np.ascontiguousarray(
            sel_flat.reshape(NCH, TT, 128, SLOTS)
                    .transpose(2, 0, 1, 3).reshape(128, -1))
        cnt_c = np.zeros((1, SLOTS), dtype=np.float32)
        nseg = seg_hi[c] - seg_lo[c]
        cnt_c[0, :nseg] = counts[seg_lo[c]:seg_hi[c]]
        in_maps.append({
            **common,
            "xt": xt_c,
            "sel": sel_c,
            "cnt": cnt_c,
        })

    global _LAST_IN_MAPS
    _LAST_IN_MAPS = in_maps
    res = bass_utils.run_bass_kernel_spmd(nc, in_maps,
                                          core_ids=list(range(N_CORES)))
    pred = np.zeros((S, O), dtype=np.float32)
    for c in range(N_CORES):
        nseg = seg_hi[c] - seg_lo[c]
        if nseg > 0:
            pred[seg_lo[c]:seg_hi[c]] = res.results[c]["pred"][:nseg]
    return pred.reshape(bs, ai, O).astype(np.float32)


_LAST_IN_MAPS = None


# revision 8
# speedup vs baseline: 1.0242x; 1.0242x over previous
"""DeepSetPred Trainium2 kernel: token encoder MLP + segment-sum + predictor
MLP on 8 NeuronCores, zero collectives.

Sharding: the host cuts the (sorted-by-segment) token axis at segment
boundaries, so every segment belongs to exactly one core. Each shard is
padded to a common length with tokens whose one-hot selector row is all
zero. Each core computes the complete segment sums for its own contiguous
range of <=SLOTS segments, runs the predictor on those rows, and writes its
private slice of the output; the host concatenates.

Structure: the encoder's third linear layer commutes with the segment sum
(it sits after the last tanh), so
    segsum(h2 @ W3 + b3) == segsum(h2) @ W3 + counts * b3
and W3 further folds into the predictor's first layer:
    enc @ P1 + pb1 == segsum(h2) @ (W3 @ P1) + counts * (b3 @ P1) + pb1.
The per-token path is only L1 + L2 + a one-hot segsum matmul over h2
(14336 PE rows per 512-token chunk). L2 is computed token-major (h1 tile
stationary, W2 moving) so the segsum needs no transpose; its bias is added
by the DVE from a broadcast tile (ACT bias is per-partition only), then ACT
applies tanh. The PE stream is skewed L1(i) | L2(i-2) | seg(i-3) so neither
the w2 weight DMA at startup nor the DVE+ACT hop ever stalls the PE. All
weights are host-pre-shuffled into dense [128, X] partition-contiguous
blocks; w1/w2 are split across the scalar+vector DMA queues to parallelize
the startup load, and xt uses 2KB partition lines.
"""

import numpy as np

import concourse.mybir as mybir
import concourse.tile as tile
from concourse import bacc
from concourse import bass_utils
from concourse.masks import make_identity

# Problem shapes (hardcoded per contract).
T, E, H, C, O = 131072, 256, 512, 256, 32
S = 128            # num segments
N_CORES = 8
TOK = 512          # tokens per chunk
MIN_SLOTS = 32     # baseline segments-per-core capacity
SG = 4             # chunks per sel DMA group
F32 = mybir.dt.float32
F32R = mybir.dt.float32r
F16 = mybir.dt.float16

EC = E // 128   # 2
HC = H // 128   # 4
TT = TOK // 128  # 4 token sub-tiles per chunk

_CACHE = {}


def _mm(nc, out, lhsT, rhs, start, stop, skip=True):
    nc.tensor.matmul(out, lhsT, rhs,
                     start=start, stop=stop, skip_group_check=skip)


def _build_nc(t_sh, SLOTS):
    assert t_sh % 128 == 0
    n_full = t_sh // TOK
    tail = t_sh - n_full * TOK
    chunks = [(i * TOK, TOK) for i in range(n_full)]
    if tail:
        chunks.append((n_full * TOK, tail))
    NCH = len(chunks)
    NSG = (NCH + SG - 1) // SG

    nc = bacc.Bacc("TRN2", target_bir_lowering=False, debug=False,
                   num_devices=N_CORES)

    # xt packed: [128, NCH, EC, TOK] -> 2KB contiguous per partition/chunk
    xt_d = nc.dram_tensor("xt", [128, NCH * EC * TOK], F16,
                          kind="ExternalInput")
    # sel packed per chunk, slot dim padded to 128 so the seg matmul keeps
    # the full 128-column stationary config (no PE col_grp switch)
    sel_d = nc.dram_tensor("sel", [128, NCH * TT * 128], F16,
                           kind="ExternalInput")
    cnt_d = nc.dram_tensor("cnt", [1, SLOTS], F32, kind="ExternalInput")
    # dense pre-shuffled weights: [128, ...] partition-major blocks
    w1_d = nc.dram_tensor("w1", [128, HC * EC * 128], F16,
                          kind="ExternalInput")      # h-major tiles
    w2_d = nc.dram_tensor("w2", [128, HC * H], F16, kind="ExternalInput")
    b1_d = nc.dram_tensor("b1", [128, HC], F32, kind="ExternalInput")
    b2f_d = nc.dram_tensor("b2f", [128, H], F32, kind="ExternalInput")
    wp1_d = nc.dram_tensor("wp1", [128, HC * HC * 128], F32,
                           kind="ExternalInput")   # W3 @ P1, k-major tiles
    b3p1_d = nc.dram_tensor("b3p1", [1, H], F32, kind="ExternalInput")
    p2_d = nc.dram_tensor("p2", [128, HC * HC * 128], F32,
                          kind="ExternalInput")
    p3_d = nc.dram_tensor("p3", [128, HC * O], F32, kind="ExternalInput")
    pb1_d = nc.dram_tensor("pb1", [128, HC], F32, kind="ExternalInput")
    pb2_d = nc.dram_tensor("pb2", [128, HC], F32, kind="ExternalInput")
    pb3_d = nc.dram_tensor("pb3", [1, O], F32, kind="ExternalInput")
    out_d = nc.dram_tensor("pred", [SLOTS, O], F32, kind="ExternalOutput")

    with tile.TileContext(nc) as tc:
        with tc.tile_pool(name="wts", bufs=1) as wp, \
             tc.tile_pool(name="xt", bufs=4) as xtp, \
             tc.tile_pool(name="sel", bufs=3) as selp, \
             tc.tile_pool(name="act", bufs=3) as actp, \
             tc.tile_pool(name="small", bufs=1) as smp, \
             tc.tile_pool(name="ps", bufs=2, space="PSUM") as psp, \
             tc.tile_pool(name="psacc", bufs=1, space="PSUM") as psa:

            # warm the ACT tanh table before the queues fill
            warm_sb = smp.tile([1, 1], F32, tag="warm", name="warm")
            nc.gpsimd.memset(warm_sb[:], 0.0)
            warm_o = smp.tile([1, 1], F32, tag="warmo", name="warmo")
            nc.scalar.activation(warm_o[:], warm_sb[:],
                                 mybir.ActivationFunctionType.Tanh)

            # ---- resident weights; every DMA is partition-contiguous.
            # w1/w2 split across the scalar+vector queues so both halves
            # land in parallel while the sync queue streams xt. ----
            # w1 (whole, 2KB lines) + b1 on the scalar ring; w2 (whole,
            # 4KB lines) leads the gpsimd ring so L2(0) is never blocked.
            w1_t = wp.tile([128, HC, EC, 128], F16, tag="w1", name="w1t")
            w1_r = w1_d.ap().rearrange("p (h e q) -> p h e q", h=HC, e=EC)
            nc.scalar.dma_start(w1_t[:], w1_r)
            b1_sb = smp.tile([128, HC], F32, tag="b1", name="b1")
            nc.scalar.dma_start(b1_sb[:], b1_d.ap())
            w2_t = wp.tile([128, HC, H], F16, tag="w2", name="w2t")
            w2_r = w2_d.ap().rearrange("p (k j) -> p k j", k=HC)
            nc.gpsimd.dma_start(w2_t[:], w2_r)
            b2f_sb = smp.tile([128, H], F32, tag="b2f", name="b2f")
            nc.gpsimd.dma_start(b2f_sb[:], b2f_d.ap())
            wp1_t = wp.tile([128, HC, HC, 128], F32R, tag="wp1", name="wp1t")
            nc.gpsimd.dma_start(
                wp1_t[:], wp1_d.ap().rearrange("p (k h q) -> p k h q",
                                               k=HC, h=HC))
            p2_t = wp.tile([128, HC, HC, 128], F32R, tag="p2", name="p2t")
            nc.gpsimd.dma_start(
                p2_t[:], p2_d.ap().rearrange("p (k h q) -> p k h q",
                                             k=HC, h=HC))
            p3_t = wp.tile([128, HC, O], F32R, tag="p3", name="p3t")
            nc.gpsimd.dma_start(
                p3_t[:], p3_d.ap().rearrange("p (k o) -> p k o", k=HC))
            b3p1row = smp.tile([1, H], F32, tag="b3p1", name="b3p1")
            nc.gpsimd.dma_start(b3p1row[:], b3p1_d.ap())
            pb1_sb = smp.tile([128, HC], F32, tag="pb1", name="pb1")
            nc.gpsimd.dma_start(pb1_sb[:], pb1_d.ap())
            pb2_sb = smp.tile([128, HC], F32, tag="pb2", name="pb2")
            nc.gpsimd.dma_start(pb2_sb[:], pb2_d.ap())
            pb3row = smp.tile([1, O], F32, tag="pb3row", name="pb3row")
            nc.gpsimd.dma_start(pb3row[:], pb3_d.ap())
            cntrow = smp.tile([1, SLOTS], F32, tag="cntrow", name="cntrow")
            nc.gpsimd.dma_start(cntrow[:], cnt_d.ap())
            ones1 = smp.tile([1, SLOTS], F32, tag="ones1", name="ones1")
            nc.gpsimd.memset(ones1[:], 1.0)
            ident = smp.tile([SLOTS, SLOTS], F32, tag="ident", name="ident")
            make_identity(nc, ident[:])

            # ---- persistent segment-sum accumulator Z[slot, h] ----
            enc_ps = psa.tile([128, H], F32, tag="encacc", name="encacc")

            xt_r = xt_d.ap().rearrange("p (c e t) -> p c e t", c=NCH, e=EC)
            sel_r = sel_d.ap().rearrange("p (c q s) -> p c q s",
                                         c=NCH, q=TT, s=128)

            sel_tiles = {}

            def dma_xt(ci):
                tok = chunks[ci][1]
                xt_t = xtp.tile([128, EC, tok], F16, tag="xt", name="xt",
                                padded_shape=[128, EC, TOK])
                nc.sync.dma_start(xt_t[:], xt_r[:, ci, :, 0:tok])
                return xt_t

            def dma_selg(g):
                lo = g * SG
                gsz = min(SG, NCH - lo)
                selg = selp.tile([128, gsz, TT, 128], F16, tag="sel",
                                 name="sel", padded_shape=[128, SG, TT,
                                                           128])
                nc.sync.dma_start(selg[:], sel_r[:, lo:lo + gsz, :, :])
                sel_tiles[g] = selg

            def l1(xt_t, tok):
                h1_t = actp.tile([128, HC, tok], F16, tag="h1", name="h1",
                                 bufs=4, padded_shape=[128, HC, TOK])
                for h in range(HC):
                    ps1 = psp.tile([128, tok], F32, tag="l1", name="l1",
                                   bufs=3, padded_shape=[128, TOK])
                    for e in range(EC):
                        _mm(nc, ps1[:], w1_t[:, h, e, :], xt_t[:, e, :],
                            start=(e == 0), stop=(e == EC - 1))
                    nc.scalar.activation(h1_t[:, h, :], ps1[:],
                                         mybir.ActivationFunctionType.Tanh,
                                         bias=b1_sb[:, h:h + 1])
                return h1_t

            def l2(h1_t, tok):
                tt = tok // 128
                h2_t = actp.tile([128, tt, H], F16, tag="h2", name="h2",
                                 padded_shape=[128, TT, H])
                for t in range(tt):
                    ps2 = psp.tile([128, H], F32, tag="l2", name="l2",
                                   bufs=3)
                    for k in range(HC):
                        _mm(nc, ps2[:], h1_t[:, k, t * 128:(t + 1) * 128],
                            w2_t[:, k, :], start=(k == 0),
                            stop=(k == HC - 1))
                    g2 = actp.tile([128, H], F16, tag="g2", name="g2")
                    nc.vector.tensor_add(g2[:], ps2[:], b2f_sb[:])
                    nc.scalar.activation(h2_t[:, t, :], g2[:],
                                         mybir.ActivationFunctionType.Tanh)
                return h2_t

            def seg(ci, h2_t, tok, is_first, is_last):
                tt = tok // 128
                selg = sel_tiles[ci // SG]
                for t in range(tt):
                    _mm(nc, enc_ps[:], selg[:, ci % SG, t, :],
                        h2_t[:, t, :],
                        start=(is_first and t == 0),
                        stop=(is_last and t == tt - 1))

            # ---- main loop: PE stream L1(i) | L2(i-2) | seg(i-3) ----
            assert NCH >= 4
            xt_q = [dma_xt(0), dma_xt(1)]
            dma_selg(0)
            h1_q = []
            h2_q = []
            for ci in range(NCH):
                if ci + 2 < NCH:
                    xt_q.append(dma_xt(ci + 2))
                    if (ci + 2) % SG == 0:
                        dma_selg((ci + 2) // SG)
                h1_q.append((l1(xt_q[ci], chunks[ci][1]), chunks[ci][1]))
                if ci >= 2:
                    h1_t, tok1 = h1_q[ci - 2]
                    h2_q.append((l2(h1_t, tok1), tok1))
                if ci >= 3:
                    h2_t, tok2 = h2_q[ci - 3]
                    seg(ci - 3, h2_t, tok2,
                        is_first=(ci == 3), is_last=False)
            # epilogue: remaining L2/seg in dependency-friendly order
            h2_q.append((l2(h1_q[NCH - 2][0], h1_q[NCH - 2][1]),
                         h1_q[NCH - 2][1]))
            seg(NCH - 3, h2_q[NCH - 3][0], h2_q[NCH - 3][1],
                is_first=False, is_last=False)
            h2_q.append((l2(h1_q[NCH - 1][0], h1_q[NCH - 1][1]),
                         h1_q[NCH - 1][1]))
            seg(NCH - 2, h2_q[NCH - 2][0], h2_q[NCH - 2][1],
                is_first=False, is_last=False)
            seg(NCH - 1, h2_q[NCH - 1][0], h2_q[NCH - 1][1],
                is_first=False, is_last=True)

            # ---- predictor on this core's own <=SLOTS segment rows ----
            # Z = segsum(h2) [SLOTS, H]; q1 = tanh(Z @ WP1 + cnt*b3p1 + pb1)
            # slice-pipelined: copy k-slice, transpose it while the next
            # slice copies, so the chain latency overlaps
            z_sb = smp.tile([SLOTS, H], F32, tag="zsb", name="zsb")
            zT = smp.tile([128, HC, SLOTS], F32R, tag="zT", name="zT")
            nc.vector.tensor_copy(z_sb[:], enc_ps[0:SLOTS, :])
            for k in range(HC):
                pst = psp.tile([128, SLOTS], F32, tag="l1", name="pst",
                               bufs=3)
                nc.tensor.transpose(pst[:], z_sb[:, k * 128:(k + 1) * 128],
                                    ident[:])
                nc.vector.tensor_copy(zT[:, k, :], pst[:])

            q1_sb = smp.tile([128, HC, SLOTS], F32R, tag="q1", name="q1")
            for h in range(HC):
                pp1 = psp.tile([128, SLOTS], F32, tag="l1", name="pp1",
                               bufs=3)
                nc.tensor.matmul(pp1[:], b3p1row[:, h * 128:(h + 1) * 128],
                                 cntrow[:], start=True, stop=False,
                                 skip_group_check=True)
                for k in range(HC):
                    _mm(nc, pp1[:], wp1_t[:, k, h, :], zT[:, k, :],
                        start=False, stop=(k == HC - 1))
                nc.scalar.activation(q1_sb[:, h, :], pp1[:],
                                     mybir.ActivationFunctionType.Tanh,
                                     bias=pb1_sb[:, h:h + 1])
            q2_sb = smp.tile([128, HC, SLOTS], F32R, tag="q2", name="q2")
            for h in range(HC):
                pp2 = psp.tile([128, SLOTS], F32, tag="l1", name="pp2",
                               bufs=3)
                for k in range(HC):
                    _mm(nc, pp2[:], p2_t[:, k, h, :], q1_sb[:, k, :],
                        start=(k == 0), stop=(k == HC - 1))
                nc.scalar.activation(q2_sb[:, h, :], pp2[:],
                                     mybir.ActivationFunctionType.Tanh,
                                     bias=pb2_sb[:, h:h + 1])

            # final: pred[slot, o] = q2.T @ P3 + pb3
            ppo = psp.tile([SLOTS, O], F32, tag="l2", name="ppo", bufs=3)
            nc.tensor.matmul(ppo[:], ones1[:], pb3row[:],
                             start=True, stop=False, skip_group_check=True)
            for k in range(HC):
                _mm(nc, ppo[:], q2_sb[:, k, :], p3_t[:, k, :],
                    start=False, stop=(k == HC - 1))
            pred_sb = smp.tile([SLOTS, O], F32, tag="pred", name="predsb")
            nc.vector.tensor_copy(pred_sb[:], ppo[:])
            nc.sync.dma_start(out_d.ap(), pred_sb[:])

    nc.compile()
    return nc


def kernel(words, seg_ids, W1, b1, W2, b2, W3, b3,
           P1, pb1, P2, pb2, P3, pb3, batch_size, alpha_iter, **_):
    words = np.asarray(words, dtype=np.float32)
    seg_ids = np.asarray(seg_ids).astype(np.int64)
    assert words.shape == (T, E), words.shape
    bs, ai = int(batch_size), int(alpha_iter)

    # --- host-side index prep: cut the sorted token axis at segment
    # boundaries so each core owns whole segments ---
    counts = np.bincount(seg_ids, minlength=S)[:S]
    starts = np.concatenate([[0], np.cumsum(counts)])   # [S+1]
    cuts = [0]
    for c in range(1, N_CORES):
        tgt = c * T // N_CORES
        j = int(np.searchsorted(starts, tgt, side="left"))
        if j > 0 and tgt - starts[j - 1] < starts[j] - tgt:
            j -= 1
        cuts.append(int(starts[j]))
    cuts.append(T)
    lens = np.diff(cuts)
    t_sh = int(np.ceil(lens.max() / 128) * 128)

    # contiguous segment range owned by each core
    seg_lo = [0] * N_CORES
    for c in range(N_CORES - 1, 0, -1):
        if lens[c] > 0:
            seg_lo[c] = int(seg_ids[cuts[c]])
        else:
            seg_lo[c] = S if c == N_CORES - 1 else seg_lo[c + 1]
    seg_hi = seg_lo[1:] + [S]
    slots_needed = max(seg_hi[c] - seg_lo[c] for c in range(N_CORES))
    SLOTS = min(128, max(MIN_SLOTS, ((slots_needed + 31) // 32) * 32))
    assert slots_needed <= SLOTS, (seg_lo, seg_hi)
    assert bs * ai == S

    xt = np.ascontiguousarray(words.T.astype(np.float16))    # [E, T] fp16
    n_full = t_sh // TOK
    tail = t_sh - n_full * TOK
    NCH = n_full + (1 if tail else 0)

    key = ("nc", t_sh, SLOTS)
    if key not in _CACHE:
        _CACHE[key] = _build_nc(t_sh, SLOTS)
    nc = _CACHE[key]

    # --- host-side weight pre-shuffles (dense [128, X] blocks) ---
    W1f, W2f = np.float32(W1), np.float32(W2)
    WP1 = (np.float32(W3) @ np.float32(P1)).astype(np.float32)  # [H, H]
    b3p1 = (np.float32(b3) @ np.float32(P1)).astype(np.float32)  # [H]

    def kmaj_tiles(Wm, dtype):
        # W [K, N] -> [128, K//128, N//128, 128] p-major
        K, N = Wm.shape
        return np.ascontiguousarray(
            Wm.reshape(K // 128, 128, N // 128, 128)
              .transpose(1, 0, 2, 3).reshape(128, -1).astype(dtype))

    common = {
        # w1 h-major: [128, HC, EC, 128]
        "w1": np.ascontiguousarray(
            W1f.astype(np.float16).reshape(EC, 128, HC, 128)
               .transpose(1, 2, 0, 3).reshape(128, -1)),
        "w2": np.ascontiguousarray(
            W2f.astype(np.float16).reshape(HC, 128, H)
               .transpose(1, 0, 2).reshape(128, -1)),
        "b1": np.ascontiguousarray(
            np.float32(b1).reshape(HC, 128).T),
        "b2f": np.ascontiguousarray(
            np.broadcast_to(np.float32(b2), (128, H))),
        "wp1": kmaj_tiles(WP1, np.float32),
        "b3p1": b3p1.reshape(1, H),
        "p2": kmaj_tiles(np.float32(P2), np.float32),
        "p3": np.ascontiguousarray(
            np.float32(P3).reshape(HC, 128, O)
              .transpose(1, 0, 2).reshape(128, -1)),
        "pb1": np.ascontiguousarray(np.float32(pb1).reshape(HC, 128).T),
        "pb2": np.ascontiguousarray(np.float32(pb2).reshape(HC, 128).T),
        "pb3": np.float32(pb3).reshape(1, O),
    }
    in_maps = []
    for c in range(N_CORES):
        lo, hi = cuts[c], cuts[c + 1]
        n = hi - lo
        # xt packed [128, NCH, EC, TOK]: 2KB lines per partition/chunk
        xt_flat = np.zeros((E, NCH * TOK), dtype=np.float16)
        xt_flat[:, :n] = xt[:, lo:hi]
        xt_c = np.ascontiguousarray(
            xt_flat.reshape(EC, 128, NCH, TOK)
                   .transpose(1, 2, 0, 3).reshape(128, -1))
        # packed one-hot selector: sel8[p, ci, q, s]
        sel_flat = np.zeros((NCH * TOK, SLOTS), dtype=np.float16)
        sel_flat[:n, :] = (seg_ids[lo:hi, None] ==
                           (seg_lo[c] + np.arange(SLOTS))[None, :])
        sel_pad = np.zeros((NCH * TOK, 128), dtype=np.float16)
        sel_pad[:, :SLOTS] = sel_flat
        sel_c = np.ascontiguousarray(
            sel_pad.reshape(NCH, TT, 128, 128)
                   .transpose(2, 0, 1, 3).reshape(128, -1))
        cnt_c = np.zeros((1, SLOTS), dtype=np.float32)
        nseg = seg_hi[c] - seg_lo[c]
        cnt_c[0, :nseg] = counts[seg_lo[c]:seg_hi[c]]
        in_maps.append({
            **common,
            "xt": xt_c,
            "sel": sel_c,
            "cnt": cnt_c,
        })

    global _LAST_IN_MAPS
    _LAST_IN_MAPS = in_maps
    res = bass_utils.run_bass_kernel_spmd(nc, in_maps,
                                          core_ids=list(range(N_CORES)))
    pred = np.zeros((S, O), dtype=np.float32)
    for c in range(N_CORES):
        nseg = seg_hi[c] - seg_lo[c]
        if nseg > 0:
            pred[seg_lo[c]:seg_hi[c]] = res.results[c]["pred"][:nseg]
    return pred.reshape(bs, ai, O).astype(np.float32)


_LAST_IN_MAPS = None


# revision 9
# speedup vs baseline: 1.0485x; 1.0237x over previous
"""DeepSetPred Trainium2 kernel: token encoder MLP + segment-sum + predictor
MLP on 8 NeuronCores, zero collectives.

Sharding: the host cuts the (sorted-by-segment) token axis at segment
boundaries, so every segment belongs to exactly one core. Each shard is
padded to a common length with tokens whose one-hot selector row is all
zero. Each core computes the complete segment sums for its own contiguous
range of <=SLOTS segments, runs the predictor on those rows, and writes its
private slice of the output; the host concatenates.

Structure: the encoder's third linear layer commutes with the segment sum
(it sits after the last tanh), so
    segsum(h2 @ W3 + b3) == segsum(h2) @ W3 + counts * b3
and W3 further folds into the predictor's first layer:
    enc @ P1 + pb1 == segsum(h2) @ (W3 @ P1) + counts * (b3 @ P1) + pb1.
The per-token path is only L1 + L2 + a one-hot segsum matmul over h2
(14336 PE rows per 512-token chunk). L2 is computed token-major (h1 tile
stationary, W2 moving) so the segsum needs no transpose; its bias is added
by the DVE from a broadcast tile (ACT bias is per-partition only), then ACT
applies tanh. The PE stream is skewed L1(i) | L2(i-2) | seg(i-3) so neither
the w2 weight DMA at startup nor the DVE+ACT hop ever stalls the PE. All
weights are host-pre-shuffled into dense [128, X] partition-contiguous
blocks; w1/w2 are split across the scalar+vector DMA queues to parallelize
the startup load, and xt uses 2KB partition lines.
"""

import numpy as np

import concourse.mybir as mybir
import concourse.tile as tile
from concourse import bacc
from concourse import bass_utils
from concourse.masks import make_identity

# Problem shapes (hardcoded per contract).
T, E, H, C, O = 131072, 256, 512, 256, 32
S = 128            # num segments
N_CORES = 8
TOK = 512          # tokens per chunk
MIN_SLOTS = 32     # baseline segments-per-core capacity
SG = 4             # chunks per sel DMA group
F32 = mybir.dt.float32
F32R = mybir.dt.float32r
F16 = mybir.dt.float16

EC = E // 128   # 2
HC = H // 128   # 4
TT = TOK // 128  # 4 token sub-tiles per chunk

_CACHE = {}


def _mm(nc, out, lhsT, rhs, start, stop, skip=True):
    nc.tensor.matmul(out, lhsT, rhs,
                     start=start, stop=stop, skip_group_check=skip)


def _build_nc(t_sh, SLOTS):
    assert t_sh % 128 == 0
    n_full = t_sh // TOK
    tail = t_sh - n_full * TOK
    chunks = [(i * TOK, TOK) for i in range(n_full)]
    if tail:
        chunks.append((n_full * TOK, tail))
    NCH = len(chunks)
    NSG = (NCH + SG - 1) // SG

    nc = bacc.Bacc("TRN2", target_bir_lowering=False, debug=False,
                   num_devices=N_CORES)

    # xt packed: [128, NCH, EC, TOK] -> 2KB contiguous per partition/chunk
    xt_d = nc.dram_tensor("xt", [128, NCH * EC * TOK], F16,
                          kind="ExternalInput")
    # sel packed per chunk, slot dim padded to 128 so the seg matmul keeps
    # the full 128-column stationary config (no PE col_grp switch)
    sel_d = nc.dram_tensor("sel", [128, NCH * TT * 128], F16,
                           kind="ExternalInput")
    cnt_d = nc.dram_tensor("cnt", [1, SLOTS], F32, kind="ExternalInput")
    # dense pre-shuffled weights: [128, ...] partition-major blocks
    w1_d = nc.dram_tensor("w1", [128, HC * EC * 128], F16,
                          kind="ExternalInput")      # h-major tiles
    w2_d = nc.dram_tensor("w2", [128, HC * H], F16, kind="ExternalInput")
    b1_d = nc.dram_tensor("b1", [128, HC], F32, kind="ExternalInput")
    b2f_d = nc.dram_tensor("b2f", [128, H], F32, kind="ExternalInput")
    wp1_d = nc.dram_tensor("wp1", [128, HC * HC * 128], F32,
                           kind="ExternalInput")   # W3 @ P1, k-major tiles
    b3p1_d = nc.dram_tensor("b3p1", [1, H], F32, kind="ExternalInput")
    p2_d = nc.dram_tensor("p2", [128, HC * HC * 128], F32,
                          kind="ExternalInput")
    p3_d = nc.dram_tensor("p3", [128, HC * O], F32, kind="ExternalInput")
    pb1_d = nc.dram_tensor("pb1", [128, HC], F32, kind="ExternalInput")
    pb2_d = nc.dram_tensor("pb2", [128, HC], F32, kind="ExternalInput")
    pb3_d = nc.dram_tensor("pb3", [1, O], F32, kind="ExternalInput")
    out_d = nc.dram_tensor("pred", [SLOTS, O], F32, kind="ExternalOutput")

    with tile.TileContext(nc) as tc:
        with tc.tile_pool(name="wts", bufs=1) as wp, \
             tc.tile_pool(name="xt", bufs=4) as xtp, \
             tc.tile_pool(name="sel", bufs=3) as selp, \
             tc.tile_pool(name="act", bufs=3) as actp, \
             tc.tile_pool(name="small", bufs=1) as smp, \
             tc.tile_pool(name="ps", bufs=2, space="PSUM") as psp, \
             tc.tile_pool(name="psacc", bufs=1, space="PSUM") as psa:

            # warm the ACT tanh table before the queues fill
            warm_sb = smp.tile([1, 1], F32, tag="warm", name="warm")
            nc.gpsimd.memset(warm_sb[:], 0.0)
            warm_o = smp.tile([1, 1], F32, tag="warmo", name="warmo")
            nc.scalar.activation(warm_o[:], warm_sb[:],
                                 mybir.ActivationFunctionType.Tanh)

            # ---- resident weights; every DMA is partition-contiguous.
            # w1/w2 split across the scalar+vector queues so both halves
            # land in parallel while the sync queue streams xt. ----
            # w1 (whole, 2KB lines) + b1 on the scalar ring; w2 (whole,
            # 4KB lines) leads the gpsimd ring so L2(0) is never blocked.
            w1_t = wp.tile([128, HC, EC, 128], F16, tag="w1", name="w1t")
            w1_r = w1_d.ap().rearrange("p (h e q) -> p h e q", h=HC, e=EC)
            nc.scalar.dma_start(w1_t[:], w1_r)
            b1_sb = smp.tile([128, HC], F32, tag="b1", name="b1")
            nc.scalar.dma_start(b1_sb[:], b1_d.ap())
            w2_t = wp.tile([128, HC, H], F16, tag="w2", name="w2t")
            w2_r = w2_d.ap().rearrange("p (k j) -> p k j", k=HC)
            nc.gpsimd.dma_start(w2_t[:], w2_r)
            b2f_sb = smp.tile([128, H], F32, tag="b2f", name="b2f")
            nc.gpsimd.dma_start(b2f_sb[:], b2f_d.ap())

            # ---- persistent segment-sum accumulator Z[slot, h] ----
            enc_ps = psa.tile([128, H], F32, tag="encacc", name="encacc")

            xt_r = xt_d.ap().rearrange("p (c e t) -> p c e t", c=NCH, e=EC)
            sel_r = sel_d.ap().rearrange("p (c q s) -> p c q s",
                                         c=NCH, q=TT, s=128)

            sel_tiles = {}

            def dma_xt(ci):
                tok = chunks[ci][1]
                xt_t = xtp.tile([128, EC, tok], F16, tag="xt", name="xt",
                                padded_shape=[128, EC, TOK])
                nc.sync.dma_start(xt_t[:], xt_r[:, ci, :, 0:tok])
                return xt_t

            def dma_selg(g):
                lo = g * SG
                gsz = min(SG, NCH - lo)
                selg = selp.tile([128, gsz, TT, 128], F16, tag="sel",
                                 name="sel", padded_shape=[128, SG, TT,
                                                           128])
                nc.gpsimd.dma_start(selg[:], sel_r[:, lo:lo + gsz, :, :])
                sel_tiles[g] = selg

            def l1(xt_t, tok):
                h1_t = actp.tile([128, HC, tok], F16, tag="h1", name="h1",
                                 bufs=4, padded_shape=[128, HC, TOK])
                for h in range(HC):
                    ps1 = psp.tile([128, tok], F32, tag="l1", name="l1",
                                   bufs=3, padded_shape=[128, TOK])
                    for e in range(EC):
                        _mm(nc, ps1[:], w1_t[:, h, e, :], xt_t[:, e, :],
                            start=(e == 0), stop=(e == EC - 1))
                    nc.scalar.activation(h1_t[:, h, :], ps1[:],
                                         mybir.ActivationFunctionType.Tanh,
                                         bias=b1_sb[:, h:h + 1])
                return h1_t

            def l2(h1_t, tok):
                tt = tok // 128
                h2_t = actp.tile([128, tt, H], F16, tag="h2", name="h2",
                                 padded_shape=[128, TT, H])
                for t in range(tt):
                    ps2 = psp.tile([128, H], F32, tag="l2", name="l2",
                                   bufs=3)
                    for k in range(HC):
                        _mm(nc, ps2[:], h1_t[:, k, t * 128:(t + 1) * 128],
                            w2_t[:, k, :], start=(k == 0),
                            stop=(k == HC - 1))
                    g2 = actp.tile([128, H], F16, tag="g2", name="g2")
                    nc.vector.tensor_add(g2[:], ps2[:], b2f_sb[:])
                    nc.scalar.activation(h2_t[:, t, :], g2[:],
                                         mybir.ActivationFunctionType.Tanh)
                return h2_t

            def seg(ci, h2_t, tok, is_first, is_last):
                tt = tok // 128
                selg = sel_tiles[ci // SG]
                for t in range(tt):
                    _mm(nc, enc_ps[:], selg[:, ci % SG, t, :],
                        h2_t[:, t, :],
                        start=(is_first and t == 0),
                        stop=(is_last and t == tt - 1))

            # ---- main loop: PE stream L1(i) | L2(i-2) | seg(i-3) ----
            assert NCH >= 4
            xt_q = [dma_xt(0), dma_xt(1)]
            dma_selg(0)
            h1_q = []
            h2_q = []
            for ci in range(NCH):
                if ci + 2 < NCH:
                    xt_q.append(dma_xt(ci + 2))
                    if (ci + 2) % SG == 0:
                        dma_selg((ci + 2) // SG)
                h1_q.append((l1(xt_q[ci], chunks[ci][1]), chunks[ci][1]))
                if ci >= 2:
                    h1_t, tok1 = h1_q[ci - 2]
                    h2_q.append((l2(h1_t, tok1), tok1))
                if ci >= 3:
                    h2_t, tok2 = h2_q[ci - 3]
                    seg(ci - 3, h2_t, tok2,
                        is_first=(ci == 3), is_last=False)
            # epilogue: remaining L2/seg in dependency-friendly order
            h2_q.append((l2(h1_q[NCH - 2][0], h1_q[NCH - 2][1]),
                         h1_q[NCH - 2][1]))
            seg(NCH - 3, h2_q[NCH - 3][0], h2_q[NCH - 3][1],
                is_first=False, is_last=False)
            h2_q.append((l2(h1_q[NCH - 1][0], h1_q[NCH - 1][1]),
                         h1_q[NCH - 1][1]))
            seg(NCH - 2, h2_q[NCH - 2][0], h2_q[NCH - 2][1],
                is_first=False, is_last=False)
            seg(NCH - 1, h2_q[NCH - 1][0], h2_q[NCH - 1][1],
                is_first=False, is_last=True)

            # ---- predictor weights (gpsimd ring, behind the sel groups;
            # needed only at the very end) ----
            wp1_t = wp.tile([128, HC, HC, 128], F32R, tag="wp1", name="wp1t")
            nc.gpsimd.dma_start(
                wp1_t[:], wp1_d.ap().rearrange("p (k h q) -> p k h q",
                                               k=HC, h=HC))
            p2_t = wp.tile([128, HC, HC, 128], F32R, tag="p2", name="p2t")
            nc.gpsimd.dma_start(
                p2_t[:], p2_d.ap().rearrange("p (k h q) -> p k h q",
                                             k=HC, h=HC))
            p3_t = wp.tile([128, HC, O], F32R, tag="p3", name="p3t")
            nc.gpsimd.dma_start(
                p3_t[:], p3_d.ap().rearrange("p (k o) -> p k o", k=HC))
            b3p1row = smp.tile([1, H], F32, tag="b3p1", name="b3p1")
            nc.gpsimd.dma_start(b3p1row[:], b3p1_d.ap())
            pb1_sb = smp.tile([128, HC], F32, tag="pb1", name="pb1")
            nc.gpsimd.dma_start(pb1_sb[:], pb1_d.ap())
            pb2_sb = smp.tile([128, HC], F32, tag="pb2", name="pb2")
            nc.gpsimd.dma_start(pb2_sb[:], pb2_d.ap())
            pb3row = smp.tile([1, O], F32, tag="pb3row", name="pb3row")
            nc.gpsimd.dma_start(pb3row[:], pb3_d.ap())
            cntrow = smp.tile([1, SLOTS], F32, tag="cntrow", name="cntrow")
            nc.gpsimd.dma_start(cntrow[:], cnt_d.ap())
            ones1 = smp.tile([1, SLOTS], F32, tag="ones1", name="ones1")
            nc.gpsimd.memset(ones1[:], 1.0)
            ident = smp.tile([SLOTS, SLOTS], F32, tag="ident", name="ident")
            make_identity(nc, ident[:])

            # ---- predictor on this core's own <=SLOTS segment rows ----
            # Z = segsum(h2) [SLOTS, H]; q1 = tanh(Z @ WP1 + cnt*b3p1 + pb1)
            # slice-pipelined: copy k-slice, transpose it while the next
            # slice copies, so the chain latency overlaps
            z_sb = smp.tile([SLOTS, H], F32, tag="zsb", name="zsb")
            zT = smp.tile([128, HC, SLOTS], F32R, tag="zT", name="zT")
            nc.vector.tensor_copy(z_sb[:], enc_ps[0:SLOTS, :])
            for k in range(HC):
                pst = psp.tile([128, SLOTS], F32, tag="l1", name="pst",
                               bufs=3)
                nc.tensor.transpose(pst[:], z_sb[:, k * 128:(k + 1) * 128],
                                    ident[:])
                nc.vector.tensor_copy(zT[:, k, :], pst[:])

            q1_sb = smp.tile([128, HC, SLOTS], F32R, tag="q1", name="q1")
            for h in range(HC):
                pp1 = psp.tile([128, SLOTS], F32, tag="l1", name="pp1",
                               bufs=3)
                nc.tensor.matmul(pp1[:], b3p1row[:, h * 128:(h + 1) * 128],
                                 cntrow[:], start=True, stop=False,
                                 skip_group_check=True)
                for k in range(HC):
                    _mm(nc, pp1[:], wp1_t[:, k, h, :], zT[:, k, :],
                        start=False, stop=(k == HC - 1))
                nc.scalar.activation(q1_sb[:, h, :], pp1[:],
                                     mybir.ActivationFunctionType.Tanh,
                                     bias=pb1_sb[:, h:h + 1])
            q2_sb = smp.tile([128, HC, SLOTS], F32R, tag="q2", name="q2")
            for h in range(HC):
                pp2 = psp.tile([128, SLOTS], F32, tag="l1", name="pp2",
                               bufs=3)
                for k in range(HC):
                    _mm(nc, pp2[:], p2_t[:, k, h, :], q1_sb[:, k, :],
                        start=(k == 0), stop=(k == HC - 1))
                nc.scalar.activation(q2_sb[:, h, :], pp2[:],
                                     mybir.ActivationFunctionType.Tanh,
                                     bias=pb2_sb[:, h:h + 1])

            # final: pred[slot, o] = q2.T @ P3 + pb3
            ppo = psp.tile([SLOTS, O], F32, tag="l2", name="ppo", bufs=3)
            nc.tensor.matmul(ppo[:], ones1[:], pb3row[:],
                             start=True, stop=False, skip_group_check=True)
            for k in range(HC):
                _mm(nc, ppo[:], q2_sb[:, k, :], p3_t[:, k, :],
                    start=False, stop=(k == HC - 1))
            pred_sb = smp.tile([SLOTS, O], F32, tag="pred", name="predsb")
            nc.vector.tensor_copy(pred_sb[:], ppo[:])
            nc.sync.dma_start(out_d.ap(), pred_sb[:])

    nc.compile()
    return nc


def kernel(words, seg_ids, W1, b1, W2, b2, W3, b3,
           P1, pb1, P2, pb2, P3, pb3, batch_size, alpha_iter, **_):
    words = np.asarray(words, dtype=np.float32)
    seg_ids = np.asarray(seg_ids).astype(np.int64)
    assert words.shape == (T, E), words.shape
    bs, ai = int(batch_size), int(alpha_iter)

    # --- host-side index prep: cut the sorted token axis at segment
    # boundaries so each core owns whole segments ---
    counts = np.bincount(seg_ids, minlength=S)[:S]
    starts = np.concatenate([[0], np.cumsum(counts)])   # [S+1]
    cuts = [0]
    for c in range(1, N_CORES):
        tgt = c * T // N_CORES
        j = int(np.searchsorted(starts, tgt, side="left"))
        if j > 0 and tgt - starts[j - 1] < starts[j] - tgt:
            j -= 1
        cuts.append(int(starts[j]))
    cuts.append(T)
    lens = np.diff(cuts)
    t_sh = int(np.ceil(lens.max() / 128) * 128)

    # contiguous segment range owned by each core
    seg_lo = [0] * N_CORES
    for c in range(N_CORES - 1, 0, -1):
        if lens[c] > 0:
            seg_lo[c] = int(seg_ids[cuts[c]])
        else:
            seg_lo[c] = S if c == N_CORES - 1 else seg_lo[c + 1]
    seg_hi = seg_lo[1:] + [S]
    slots_needed = max(seg_hi[c] - seg_lo[c] for c in range(N_CORES))
    SLOTS = min(128, max(MIN_SLOTS, ((slots_needed + 31) // 32) * 32))
    assert slots_needed <= SLOTS, (seg_lo, seg_hi)
    assert bs * ai == S

    xt = np.ascontiguousarray(words.T.astype(np.float16))    # [E, T] fp16
    n_full = t_sh // TOK
    tail = t_sh - n_full * TOK
    NCH = n_full + (1 if tail else 0)

    key = ("nc", t_sh, SLOTS)
    if key not in _CACHE:
        _CACHE[key] = _build_nc(t_sh, SLOTS)
    nc = _CACHE[key]

    # --- host-side weight pre-shuffles (dense [128, X] blocks) ---
    W1f, W2f = np.float32(W1), np.float32(W2)
    WP1 = (np.float32(W3) @ np.float32(P1)).astype(np.float32)  # [H, H]
    b3p1 = (np.float32(b3) @ np.float32(P1)).astype(np.float32)  # [H]

    def kmaj_tiles(Wm, dtype):
        # W [K, N] -> [128, K//128, N//128, 128] p-major
        K, N = Wm.shape
        return np.ascontiguousarray(
            Wm.reshape(K // 128, 128, N // 128, 128)
              .transpose(1, 0, 2, 3).reshape(128, -1).astype(dtype))

    common = {
        # w1 h-major: [128, HC, EC, 128]
        "w1": np.ascontiguousarray(
            W1f.astype(np.float16).reshape(EC, 128, HC, 128)
               .transpose(1, 2, 0, 3).reshape(128, -1)),
        "w2": np.ascontiguousarray(
            W2f.astype(np.float16).reshape(HC, 128, H)
               .transpose(1, 0, 2).reshape(128, -1)),
        "b1": np.ascontiguousarray(
            np.float32(b1).reshape(HC, 128).T),
        "b2f": np.ascontiguousarray(
            np.broadcast_to(np.float32(b2), (128, H))),
        "wp1": kmaj_tiles(WP1, np.float32),
        "b3p1": b3p1.reshape(1, H),
        "p2": kmaj_tiles(np.float32(P2), np.float32),
        "p3": np.ascontiguousarray(
            np.float32(P3).reshape(HC, 128, O)
              .transpose(1, 0, 2).reshape(128, -1)),
        "pb1": np.ascontiguousarray(np.float32(pb1).reshape(HC, 128).T),
        "pb2": np.ascontiguousarray(np.float32(pb2).reshape(HC, 128).T),
        "pb3": np.float32(pb3).reshape(1, O),
    }
    in_maps = []
    for c in range(N_CORES):
        lo, hi = cuts[c], cuts[c + 1]
        n = hi - lo
        # xt packed [128, NCH, EC, TOK]: 2KB lines per partition/chunk
        xt_flat = np.zeros((E, NCH * TOK), dtype=np.float16)
        xt_flat[:, :n] = xt[:, lo:hi]
        xt_c = np.ascontiguousarray(
            xt_flat.reshape(EC, 128, NCH, TOK)
                   .transpose(1, 2, 0, 3).reshape(128, -1))
        # packed one-hot selector: sel8[p, ci, q, s]
        sel_flat = np.zeros((NCH * TOK, SLOTS), dtype=np.float16)
        sel_flat[:n, :] = (seg_ids[lo:hi, None] ==
                           (seg_lo[c] + np.arange(SLOTS))[None, :])
        sel_pad = np.zeros((NCH * TOK, 128), dtype=np.float16)
        sel_pad[:, :SLOTS] = sel_flat
        sel_c = np.ascontiguousarray(
            sel_pad.reshape(NCH, TT, 128, 128)
                   .transpose(2, 0, 1, 3).reshape(128, -1))
        cnt_c = np.zeros((1, SLOTS), dtype=np.float32)
        nseg = seg_hi[c] - seg_lo[c]
        cnt_c[0, :nseg] = counts[seg_lo[c]:seg_hi[c]]
        in_maps.append({
            **common,
            "xt": xt_c,
            "sel": sel_c,
            "cnt": cnt_c,
        })

    global _LAST_IN_MAPS
    _LAST_IN_MAPS = in_maps
    res = bass_utils.run_bass_kernel_spmd(nc, in_maps,
                                          core_ids=list(range(N_CORES)))
    pred = np.zeros((S, O), dtype=np.float32)
    for c in range(N_CORES):
        nseg = seg_hi[c] - seg_lo[c]
        if nseg > 0:
            pred[seg_lo[c]:seg_hi[c]] = res.results[c]["pred"][:nseg]
    return pred.reshape(bs, ai, O).astype(np.float32)


_LAST_IN_MAPS = None


# revision 10
# speedup vs baseline: 1.0590x; 1.0101x over previous
"""DeepSetPred Trainium2 kernel: token encoder MLP + segment-sum + predictor
MLP on 8 NeuronCores, zero collectives.

Sharding: the host cuts the (sorted-by-segment) token axis at segment
boundaries, so every segment belongs to exactly one core. Each shard is
padded to a common length with tokens whose one-hot selector row is all
zero. Each core computes the complete segment sums for its own contiguous
range of <=SLOTS segments, runs the predictor on those rows, and writes its
private slice of the output; the host concatenates.

Structure: the encoder's third linear layer commutes with the segment sum
(it sits after the last tanh), so
    segsum(h2 @ W3 + b3) == segsum(h2) @ W3 + counts * b3
and W3 further folds into the predictor's first layer:
    enc @ P1 + pb1 == segsum(h2) @ (W3 @ P1) + counts * (b3 @ P1) + pb1.
The per-token path is only L1 + L2 + a one-hot segsum matmul over h2
(14336 PE rows per 512-token chunk). L2 is computed token-major (h1 tile
stationary, W2 moving) so the segsum needs no transpose; its bias is added
by the DVE from a broadcast tile (ACT bias is per-partition only), then ACT
applies tanh. The PE stream is skewed L1(i) | L2(i-2) | seg(i-3) so neither
the w2 weight DMA at startup nor the DVE+ACT hop ever stalls the PE. All
weights are host-pre-shuffled into dense [128, X] partition-contiguous
blocks; w1/w2 are split across the scalar+vector DMA queues to parallelize
the startup load, and xt uses 2KB partition lines.
"""

import numpy as np

import concourse.mybir as mybir
import concourse.tile as tile
from concourse import bacc
from concourse import bass_utils
from concourse.masks import make_identity

# Problem shapes (hardcoded per contract).
T, E, H, C, O = 131072, 256, 512, 256, 32
S = 128            # num segments
N_CORES = 8
TOK = 512          # tokens per chunk
MIN_SLOTS = 32     # baseline segments-per-core capacity
SG = 4             # chunks per sel DMA group
F32 = mybir.dt.float32
F32R = mybir.dt.float32r
F16 = mybir.dt.float16

EC = E // 128   # 2
HC = H // 128   # 4
TT = TOK // 128  # 4 token sub-tiles per chunk

_CACHE = {}


def _mm(nc, out, lhsT, rhs, start, stop, skip=True):
    nc.tensor.matmul(out, lhsT, rhs,
                     start=start, stop=stop, skip_group_check=skip)


def _build_nc(t_sh, SLOTS):
    assert t_sh % 128 == 0
    n_full = t_sh // TOK
    tail = t_sh - n_full * TOK
    chunks = [(i * TOK, TOK) for i in range(n_full)]
    if tail:
        chunks.append((n_full * TOK, tail))
    NCH = len(chunks)
    NSG = (NCH + SG - 1) // SG

    nc = bacc.Bacc("TRN2", target_bir_lowering=False, debug=False,
                   num_devices=N_CORES)

    # xt packed: [128, NCH, EC, TOK] -> 2KB contiguous per partition/chunk
    xt_d = nc.dram_tensor("xt", [128, NCH * EC * TOK], F16,
                          kind="ExternalInput")
    # sel packed per chunk, slot dim padded to 128 so the seg matmul keeps
    # the full 128-column stationary config (no PE col_grp switch)
    sel_d = nc.dram_tensor("sel", [128, NCH * TT * 128], F16,
                           kind="ExternalInput")
    cnt_d = nc.dram_tensor("cnt", [1, SLOTS], F32, kind="ExternalInput")
    # dense pre-shuffled weights: [128, ...] partition-major blocks
    w1_d = nc.dram_tensor("w1", [128, HC * EC * 128], F16,
                          kind="ExternalInput")      # h-major tiles
    w2_d = nc.dram_tensor("w2", [128, HC * H], F16, kind="ExternalInput")
    b1_d = nc.dram_tensor("b1", [128, HC], F32, kind="ExternalInput")
    b2f_d = nc.dram_tensor("b2f", [128, H], F32, kind="ExternalInput")
    wp1_d = nc.dram_tensor("wp1", [128, HC * HC * 128], F16,
                           kind="ExternalInput")   # W3 @ P1, k-major tiles
    b3p1_d = nc.dram_tensor("b3p1", [1, H], F32, kind="ExternalInput")
    p2_d = nc.dram_tensor("p2", [128, HC * HC * 128], F16,
                          kind="ExternalInput")
    p3_d = nc.dram_tensor("p3", [128, HC * O], F16, kind="ExternalInput")
    pb1_d = nc.dram_tensor("pb1", [128, HC], F32, kind="ExternalInput")
    pb2_d = nc.dram_tensor("pb2", [128, HC], F32, kind="ExternalInput")
    pb3_d = nc.dram_tensor("pb3", [1, O], F32, kind="ExternalInput")
    out_d = nc.dram_tensor("pred", [SLOTS, O], F32, kind="ExternalOutput")

    with tile.TileContext(nc) as tc:
        with tc.tile_pool(name="wts", bufs=1) as wp, \
             tc.tile_pool(name="xt", bufs=4) as xtp, \
             tc.tile_pool(name="sel", bufs=3) as selp, \
             tc.tile_pool(name="act", bufs=3) as actp, \
             tc.tile_pool(name="small", bufs=1) as smp, \
             tc.tile_pool(name="ps", bufs=2, space="PSUM") as psp, \
             tc.tile_pool(name="psacc", bufs=1, space="PSUM") as psa:

            # warm the ACT tanh table before the queues fill
            warm_sb = smp.tile([1, 1], F32, tag="warm", name="warm")
            nc.gpsimd.memset(warm_sb[:], 0.0)
            warm_o = smp.tile([1, 1], F32, tag="warmo", name="warmo")
            nc.scalar.activation(warm_o[:], warm_sb[:],
                                 mybir.ActivationFunctionType.Tanh)

            # ---- resident weights; every DMA is partition-contiguous.
            # w1/w2 split across the scalar+vector queues so both halves
            # land in parallel while the sync queue streams xt. ----
            # w1 (whole, 2KB lines) + b1 on the scalar ring; w2 (whole,
            # 4KB lines) leads the gpsimd ring so L2(0) is never blocked.
            w1_t = wp.tile([128, HC, EC, 128], F16, tag="w1", name="w1t")
            w1_r = w1_d.ap().rearrange("p (h e q) -> p h e q", h=HC, e=EC)
            nc.scalar.dma_start(w1_t[:], w1_r)
            b1_sb = smp.tile([128, HC], F32, tag="b1", name="b1")
            nc.scalar.dma_start(b1_sb[:], b1_d.ap())
            w2_t = wp.tile([128, HC, H], F16, tag="w2", name="w2t")
            w2_r = w2_d.ap().rearrange("p (k j) -> p k j", k=HC)
            nc.gpsimd.dma_start(w2_t[:], w2_r)
            b2f_sb = smp.tile([128, H], F32, tag="b2f", name="b2f")
            nc.gpsimd.dma_start(b2f_sb[:], b2f_d.ap())

            # ---- persistent segment-sum accumulator Z[slot, h] ----
            enc_ps = psa.tile([128, H], F32, tag="encacc", name="encacc")

            xt_r = xt_d.ap().rearrange("p (c e t) -> p c e t", c=NCH, e=EC)
            sel_r = sel_d.ap().rearrange("p (c q s) -> p c q s",
                                         c=NCH, q=TT, s=128)

            sel_tiles = {}

            def dma_xt(ci):
                tok = chunks[ci][1]
                xt_t = xtp.tile([128, EC, tok], F16, tag="xt", name="xt",
                                padded_shape=[128, EC, TOK])
                nc.sync.dma_start(xt_t[:], xt_r[:, ci, :, 0:tok])
                return xt_t

            def dma_selg(g):
                lo = g * SG
                gsz = min(SG, NCH - lo)
                selg = selp.tile([128, gsz, TT, 128], F16, tag="sel",
                                 name="sel", padded_shape=[128, SG, TT,
                                                           128])
                nc.gpsimd.dma_start(selg[:], sel_r[:, lo:lo + gsz, :, :])
                sel_tiles[g] = selg

            def l1(xt_t, tok):
                h1_t = actp.tile([128, HC, tok], F16, tag="h1", name="h1",
                                 bufs=4, padded_shape=[128, HC, TOK])
                for h in range(HC):
                    ps1 = psp.tile([128, tok], F32, tag="l1", name="l1",
                                   bufs=3, padded_shape=[128, TOK])
                    for e in range(EC):
                        _mm(nc, ps1[:], w1_t[:, h, e, :], xt_t[:, e, :],
                            start=(e == 0), stop=(e == EC - 1))
                    nc.scalar.activation(h1_t[:, h, :], ps1[:],
                                         mybir.ActivationFunctionType.Tanh,
                                         bias=b1_sb[:, h:h + 1])
                return h1_t

            def l2(h1_t, tok):
                tt = tok // 128
                h2_t = actp.tile([128, tt, H], F16, tag="h2", name="h2",
                                 padded_shape=[128, TT, H])
                for t in range(tt):
                    ps2 = psp.tile([128, H], F32, tag="l2", name="l2",
                                   bufs=3)
                    for k in range(HC):
                        _mm(nc, ps2[:], h1_t[:, k, t * 128:(t + 1) * 128],
                            w2_t[:, k, :], start=(k == 0),
                            stop=(k == HC - 1))
                    g2 = actp.tile([128, H], F16, tag="g2", name="g2")
                    nc.vector.tensor_add(g2[:], ps2[:], b2f_sb[:])
                    nc.scalar.activation(h2_t[:, t, :], g2[:],
                                         mybir.ActivationFunctionType.Tanh)
                return h2_t

            def seg(ci, h2_t, tok, is_first, is_last):
                tt = tok // 128
                selg = sel_tiles[ci // SG]
                for t in range(tt):
                    _mm(nc, enc_ps[:], selg[:, ci % SG, t, :],
                        h2_t[:, t, :],
                        start=(is_first and t == 0),
                        stop=(is_last and t == tt - 1))

            # ---- main loop: PE stream L1(i) | L2(i-2) | seg(i-3) ----
            assert NCH >= 4
            xt_q = [dma_xt(0), dma_xt(1)]
            dma_selg(0)
            h1_q = []
            h2_q = []
            for ci in range(NCH):
                if ci + 2 < NCH:
                    xt_q.append(dma_xt(ci + 2))
                    if (ci + 2) % SG == 0:
                        dma_selg((ci + 2) // SG)
                h1_q.append((l1(xt_q[ci], chunks[ci][1]), chunks[ci][1]))
                if ci >= 2:
                    h1_t, tok1 = h1_q[ci - 2]
                    h2_q.append((l2(h1_t, tok1), tok1))
                if ci >= 3:
                    h2_t, tok2 = h2_q[ci - 3]
                    seg(ci - 3, h2_t, tok2,
                        is_first=(ci == 3), is_last=False)
            # epilogue: both remaining L2 phases first, then the three
            # remaining seg phases (maximizes cover for the DVE+ACT chains)
            h2_q.append((l2(h1_q[NCH - 2][0], h1_q[NCH - 2][1]),
                         h1_q[NCH - 2][1]))
            h2_q.append((l2(h1_q[NCH - 1][0], h1_q[NCH - 1][1]),
                         h1_q[NCH - 1][1]))
            for sc in range(NCH - 3, NCH):
                seg(sc, h2_q[sc][0], h2_q[sc][1],
                    is_first=False, is_last=(sc == NCH - 1))

            # ---- predictor weights (gpsimd ring, behind the sel groups;
            # needed only at the very end) ----
            wp1_t = wp.tile([128, HC, HC, 128], F16, tag="wp1", name="wp1t")
            nc.gpsimd.dma_start(
                wp1_t[:], wp1_d.ap().rearrange("p (k h q) -> p k h q",
                                               k=HC, h=HC))
            p2_t = wp.tile([128, HC, HC, 128], F16, tag="p2", name="p2t")
            nc.gpsimd.dma_start(
                p2_t[:], p2_d.ap().rearrange("p (k h q) -> p k h q",
                                             k=HC, h=HC))
            p3_t = wp.tile([128, HC, O], F16, tag="p3", name="p3t")
            nc.gpsimd.dma_start(
                p3_t[:], p3_d.ap().rearrange("p (k o) -> p k o", k=HC))
            b3p1row = smp.tile([1, H], F32, tag="b3p1", name="b3p1")
            nc.gpsimd.dma_start(b3p1row[:], b3p1_d.ap())
            pb1_sb = smp.tile([128, HC], F32, tag="pb1", name="pb1")
            nc.gpsimd.dma_start(pb1_sb[:], pb1_d.ap())
            pb2_sb = smp.tile([128, HC], F32, tag="pb2", name="pb2")
            nc.gpsimd.dma_start(pb2_sb[:], pb2_d.ap())
            pb3row = smp.tile([1, O], F32, tag="pb3row", name="pb3row")
            nc.gpsimd.dma_start(pb3row[:], pb3_d.ap())
            cntrow = smp.tile([1, SLOTS], F32, tag="cntrow", name="cntrow")
            nc.gpsimd.dma_start(cntrow[:], cnt_d.ap())
            ones1 = smp.tile([1, SLOTS], F32, tag="ones1", name="ones1")
            nc.gpsimd.memset(ones1[:], 1.0)
            ident = smp.tile([SLOTS, SLOTS], F16, tag="ident", name="ident")
            make_identity(nc, ident[:])

            # ---- predictor on this core's own <=SLOTS segment rows ----
            # Z = segsum(h2) [SLOTS, H]; q1 = tanh(Z @ WP1 + cnt*b3p1 + pb1)
            # slice-pipelined: copy k-slice, transpose it while the next
            # slice copies, so the chain latency overlaps
            z_sb = smp.tile([SLOTS, H], F16, tag="zsb", name="zsb")
            zT = smp.tile([128, HC, SLOTS], F16, tag="zT", name="zT")
            psts = []
            nc.vector.tensor_copy(z_sb[:, 0:128], enc_ps[0:SLOTS, 0:128])
            for k in range(HC):
                if k + 1 < HC:
                    nc.vector.tensor_copy(
                        z_sb[:, (k + 1) * 128:(k + 2) * 128],
                        enc_ps[0:SLOTS, (k + 1) * 128:(k + 2) * 128])
                pst = psp.tile([128, SLOTS], F16, tag="l1", name="pst",
                               bufs=3)
                nc.tensor.transpose(pst[:], z_sb[:, k * 128:(k + 1) * 128],
                                    ident[:])
                nc.vector.tensor_copy(zT[:, k, :], pst[:])

            q1_sb = smp.tile([128, HC, SLOTS], F16, tag="q1", name="q1")
            for h in range(HC):
                pp1 = psp.tile([128, SLOTS], F32, tag="l1", name="pp1",
                               bufs=3)
                nc.tensor.matmul(pp1[:], b3p1row[:, h * 128:(h + 1) * 128],
                                 cntrow[:], start=True, stop=False,
                                 skip_group_check=True)
                for k in range(HC):
                    _mm(nc, pp1[:], wp1_t[:, k, h, :], zT[:, k, :],
                        start=False, stop=(k == HC - 1))
                nc.scalar.activation(q1_sb[:, h, :], pp1[:],
                                     mybir.ActivationFunctionType.Tanh,
                                     bias=pb1_sb[:, h:h + 1])
            q2_sb = smp.tile([128, HC, SLOTS], F16, tag="q2", name="q2")
            for h in range(HC):
                pp2 = psp.tile([128, SLOTS], F32, tag="l1", name="pp2",
                               bufs=3)
                for k in range(HC):
                    _mm(nc, pp2[:], p2_t[:, k, h, :], q1_sb[:, k, :],
                        start=(k == 0), stop=(k == HC - 1))
                nc.scalar.activation(q2_sb[:, h, :], pp2[:],
                                     mybir.ActivationFunctionType.Tanh,
                                     bias=pb2_sb[:, h:h + 1])

            # final: pred[slot, o] = q2.T @ P3 + pb3
            ppo = psp.tile([SLOTS, O], F32, tag="l2", name="ppo", bufs=3)
            nc.tensor.matmul(ppo[:], ones1[:], pb3row[:],
                             start=True, stop=False, skip_group_check=True)
            for k in range(HC):
                _mm(nc, ppo[:], q2_sb[:, k, :], p3_t[:, k, :],
                    start=False, stop=(k == HC - 1))
            pred_sb = smp.tile([SLOTS, O], F32, tag="pred", name="predsb")
            nc.vector.tensor_copy(pred_sb[:], ppo[:])
            nc.sync.dma_start(out_d.ap(), pred_sb[:])

    nc.compile()
    return nc


def kernel(words, seg_ids, W1, b1, W2, b2, W3, b3,
           P1, pb1, P2, pb2, P3, pb3, batch_size, alpha_iter, **_):
    words = np.asarray(words, dtype=np.float32)
    seg_ids = np.asarray(seg_ids).astype(np.int64)
    assert words.shape == (T, E), words.shape
    bs, ai = int(batch_size), int(alpha_iter)

    # --- host-side index prep: cut the sorted token axis at segment
    # boundaries so each core owns whole segments ---
    counts = np.bincount(seg_ids, minlength=S)[:S]
    starts = np.concatenate([[0], np.cumsum(counts)])   # [S+1]
    cuts = [0]
    for c in range(1, N_CORES):
        tgt = c * T // N_CORES
        j = int(np.searchsorted(starts, tgt, side="left"))
        if j > 0 and tgt - starts[j - 1] < starts[j] - tgt:
            j -= 1
        cuts.append(int(starts[j]))
    cuts.append(T)
    lens = np.diff(cuts)
    t_sh = int(np.ceil(lens.max() / 128) * 128)

    # contiguous segment range owned by each core
    seg_lo = [0] * N_CORES
    for c in range(N_CORES - 1, 0, -1):
        if lens[c] > 0:
            seg_lo[c] = int(seg_ids[cuts[c]])
        else:
            seg_lo[c] = S if c == N_CORES - 1 else seg_lo[c + 1]
    seg_hi = seg_lo[1:] + [S]
    slots_needed = max(seg_hi[c] - seg_lo[c] for c in range(N_CORES))
    SLOTS = min(128, max(MIN_SLOTS, ((slots_needed + 31) // 32) * 32))
    assert slots_needed <= SLOTS, (seg_lo, seg_hi)
    assert bs * ai == S

    xt = np.ascontiguousarray(words.T.astype(np.float16))    # [E, T] fp16
    n_full = t_sh // TOK
    tail = t_sh - n_full * TOK
    NCH = n_full + (1 if tail else 0)

    key = ("nc", t_sh, SLOTS)
    if key not in _CACHE:
        _CACHE[key] = _build_nc(t_sh, SLOTS)
    nc = _CACHE[key]

    # --- host-side weight pre-shuffles (dense [128, X] blocks) ---
    W1f, W2f = np.float32(W1), np.float32(W2)
    WP1 = (np.float32(W3) @ np.float32(P1)).astype(np.float32)  # [H, H]
    b3p1 = (np.float32(b3) @ np.float32(P1)).astype(np.float32)  # [H]

    def kmaj_tiles(Wm, dtype):
        # W [K, N] -> [128, K//128, N//128, 128] p-major
        K, N = Wm.shape
        return np.ascontiguousarray(
            Wm.reshape(K // 128, 128, N // 128, 128)
              .transpose(1, 0, 2, 3).reshape(128, -1).astype(dtype))

    common = {
        # w1 h-major: [128, HC, EC, 128]
        "w1": np.ascontiguousarray(
            W1f.astype(np.float16).reshape(EC, 128, HC, 128)
               .transpose(1, 2, 0, 3).reshape(128, -1)),
        "w2": np.ascontiguousarray(
            W2f.astype(np.float16).reshape(HC, 128, H)
               .transpose(1, 0, 2).reshape(128, -1)),
        "b1": np.ascontiguousarray(
            np.float32(b1).reshape(HC, 128).T),
        "b2f": np.ascontiguousarray(
            np.broadcast_to(np.float32(b2), (128, H))),
        "wp1": kmaj_tiles(WP1, np.float16),
        "b3p1": b3p1.reshape(1, H),
        "p2": kmaj_tiles(np.float32(P2), np.float16),
        "p3": np.ascontiguousarray(
            np.float16(P3).reshape(HC, 128, O)
              .transpose(1, 0, 2).reshape(128, -1)),
        "pb1": np.ascontiguousarray(np.float32(pb1).reshape(HC, 128).T),
        "pb2": np.ascontiguousarray(np.float32(pb2).reshape(HC, 128).T),
        "pb3": np.float32(pb3).reshape(1, O),
    }
    in_maps = []
    for c in range(N_CORES):
        lo, hi = cuts[c], cuts[c + 1]
        n = hi - lo
        # xt packed [128, NCH, EC, TOK]: 2KB lines per partition/chunk
        xt_flat = np.zeros((E, NCH * TOK), dtype=np.float16)
        xt_flat[:, :n] = xt[:, lo:hi]
        xt_c = np.ascontiguousarray(
            xt_flat.reshape(EC, 128, NCH, TOK)
                   .transpose(1, 2, 0, 3).reshape(128, -1))
        # packed one-hot selector: sel8[p, ci, q, s]
        sel_flat = np.zeros((NCH * TOK, SLOTS), dtype=np.float16)
        sel_flat[:n, :] = (seg_ids[lo:hi, None] ==
                           (seg_lo[c] + np.arange(SLOTS))[None, :])
        sel_pad = np.zeros((NCH * TOK, 128), dtype=np.float16)
        sel_pad[:, :SLOTS] = sel_flat
        sel_c = np.ascontiguousarray(
            sel_pad.reshape(NCH, TT, 128, 128)
                   .transpose(2, 0, 1, 3).reshape(128, -1))
        cnt_c = np.zeros((1, SLOTS), dtype=np.float32)
        nseg = seg_hi[c] - seg_lo[c]
        cnt_c[0, :nseg] = counts[seg_lo[c]:seg_hi[c]]
        in_maps.append({
            **common,
            "xt": xt_c,
            "sel": sel_c,
            "cnt": cnt_c,
        })

    global _LAST_IN_MAPS
    _LAST_IN_MAPS = in_maps
    res = bass_utils.run_bass_kernel_spmd(nc, in_maps,
                                          core_ids=list(range(N_CORES)))
    pred = np.zeros((S, O), dtype=np.float32)
    for c in range(N_CORES):
        nseg = seg_hi[c] - seg_lo[c]
        if nseg > 0:
            pred[seg_lo[c]:seg_hi[c]] = res.results[c]["pred"][:nseg]
    return pred.reshape(bs, ai, O).astype(np.float32)


_LAST_IN_MAPS = None


# revision 11
# speedup vs baseline: 1.0636x; 1.0043x over previous
"""DeepSetPred Trainium2 kernel: token encoder MLP + segment-sum + predictor
MLP on 8 NeuronCores, zero collectives.

Sharding: the host cuts the (sorted-by-segment) token axis at segment
boundaries, so every segment belongs to exactly one core. Each shard is
padded to a common length with tokens whose one-hot selector row is all
zero. Each core computes the complete segment sums for its own contiguous
range of <=SLOTS segments, runs the predictor on those rows, and writes its
private slice of the output; the host concatenates.

Structure: the encoder's third linear layer commutes with the segment sum
(it sits after the last tanh), so
    segsum(h2 @ W3 + b3) == segsum(h2) @ W3 + counts * b3
and W3 further folds into the predictor's first layer:
    enc @ P1 + pb1 == segsum(h2) @ (W3 @ P1) + counts * (b3 @ P1) + pb1.
The per-token path is only L1 + L2 + a one-hot segsum matmul over h2
(14336 PE rows per 512-token chunk). L2 is computed token-major (h1 tile
stationary, W2 moving) so the segsum needs no transpose; its bias is added
by the DVE from a broadcast tile (ACT bias is per-partition only), then ACT
applies tanh. The PE stream is skewed L1(i) | L2(i-2) | seg(i-3) so neither
the w2 weight DMA at startup nor the DVE+ACT hop ever stalls the PE. All
weights are host-pre-shuffled into dense [128, X] partition-contiguous
blocks; w1/w2 are split across the scalar+vector DMA queues to parallelize
the startup load, and xt uses 2KB partition lines.
"""

import numpy as np

import concourse.mybir as mybir
import concourse.tile as tile
from concourse import bacc
from concourse import bass_utils
from concourse.masks import make_identity

# Problem shapes (hardcoded per contract).
T, E, H, C, O = 131072, 256, 512, 256, 32
S = 128            # num segments
N_CORES = 8
TOK = 512          # tokens per chunk
MIN_SLOTS = 32     # baseline segments-per-core capacity
SG = 4             # chunks per sel DMA group
F32 = mybir.dt.float32
F32R = mybir.dt.float32r
F16 = mybir.dt.float16

EC = E // 128   # 2
HC = H // 128   # 4
TT = TOK // 128  # 4 token sub-tiles per chunk

_CACHE = {}


def _mm(nc, out, lhsT, rhs, start, stop, skip=True):
    nc.tensor.matmul(out, lhsT, rhs,
                     start=start, stop=stop, skip_group_check=skip)


def _build_nc(t_sh, SLOTS):
    assert t_sh % 128 == 0
    n_full = t_sh // TOK
    tail = t_sh - n_full * TOK
    chunks = [(i * TOK, TOK) for i in range(n_full)]
    if tail:
        chunks.append((n_full * TOK, tail))
    NCH = len(chunks)
    NSG = (NCH + SG - 1) // SG

    nc = bacc.Bacc("TRN2", target_bir_lowering=False, debug=False,
                   num_devices=N_CORES)

    # xt packed: [128, NCH, EC, TOK] -> 2KB contiguous per partition/chunk
    xt_d = nc.dram_tensor("xt", [128, NCH * EC * TOK], F16,
                          kind="ExternalInput")
    # sel packed per chunk, slot dim padded to 128 so the seg matmul keeps
    # the full 128-column stationary config (no PE col_grp switch)
    sel_d = nc.dram_tensor("sel", [128, NCH * TT * 128], F16,
                           kind="ExternalInput")
    cnt_d = nc.dram_tensor("cnt", [1, SLOTS], F32, kind="ExternalInput")
    # dense pre-shuffled weights: [128, ...] partition-major blocks
    w1_d = nc.dram_tensor("w1", [128, HC * EC * 128], F16,
                          kind="ExternalInput")      # h-major tiles
    w2_d = nc.dram_tensor("w2", [128, HC * H], F16, kind="ExternalInput")
    b1_d = nc.dram_tensor("b1", [128, HC], F32, kind="ExternalInput")
    b2f_d = nc.dram_tensor("b2f", [128, H], F32, kind="ExternalInput")
    wp1_d = nc.dram_tensor("wp1", [128, HC * HC * 128], F16,
                           kind="ExternalInput")   # W3 @ P1, k-major tiles
    b3p1_d = nc.dram_tensor("b3p1", [1, H], F32, kind="ExternalInput")
    p2_d = nc.dram_tensor("p2", [128, HC * HC * 128], F16,
                          kind="ExternalInput")
    p3_d = nc.dram_tensor("p3", [128, HC * O], F16, kind="ExternalInput")
    pb1_d = nc.dram_tensor("pb1", [128, HC], F32, kind="ExternalInput")
    pb2_d = nc.dram_tensor("pb2", [128, HC], F32, kind="ExternalInput")
    pb3_d = nc.dram_tensor("pb3", [1, O], F32, kind="ExternalInput")
    out_d = nc.dram_tensor("pred", [SLOTS, O], F32, kind="ExternalOutput")

    with tile.TileContext(nc) as tc:
        with tc.tile_pool(name="wts", bufs=1) as wp, \
             tc.tile_pool(name="xt", bufs=5) as xtp, \
             tc.tile_pool(name="sel", bufs=3) as selp, \
             tc.tile_pool(name="act", bufs=3) as actp, \
             tc.tile_pool(name="small", bufs=1) as smp, \
             tc.tile_pool(name="ps", bufs=2, space="PSUM") as psp, \
             tc.tile_pool(name="psacc", bufs=1, space="PSUM") as psa:

            # warm the ACT tanh table before the queues fill
            warm_sb = smp.tile([1, 1], F32, tag="warm", name="warm")
            nc.gpsimd.memset(warm_sb[:], 0.0)
            warm_o = smp.tile([1, 1], F32, tag="warmo", name="warmo")
            nc.scalar.activation(warm_o[:], warm_sb[:],
                                 mybir.ActivationFunctionType.Tanh)

            # ---- resident weights; every DMA is partition-contiguous.
            # w1/w2 split across the scalar+vector queues so both halves
            # land in parallel while the sync queue streams xt. ----
            # w1 (whole, 2KB lines) + b1 on the scalar ring; w2 (whole,
            # 4KB lines) leads the gpsimd ring so L2(0) is never blocked.
            w1_t = wp.tile([128, HC, EC, 128], F16, tag="w1", name="w1t")
            w1_r = w1_d.ap().rearrange("p (h e q) -> p h e q", h=HC, e=EC)
            nc.scalar.dma_start(w1_t[:], w1_r)
            b1_sb = smp.tile([128, HC], F32, tag="b1", name="b1")
            nc.scalar.dma_start(b1_sb[:], b1_d.ap())
            w2_t = wp.tile([128, HC, H], F16, tag="w2", name="w2t")
            w2_r = w2_d.ap().rearrange("p (k j) -> p k j", k=HC)
            nc.gpsimd.dma_start(w2_t[:], w2_r)
            b2f_sb = smp.tile([128, H], F32, tag="b2f", name="b2f")
            nc.gpsimd.dma_start(b2f_sb[:], b2f_d.ap())

            # ---- persistent segment-sum accumulator Z[slot, h] ----
            enc_ps = psa.tile([128, H], F32, tag="encacc", name="encacc")

            xt_r = xt_d.ap().rearrange("p (c e t) -> p c e t", c=NCH, e=EC)
            sel_r = sel_d.ap().rearrange("p (c q s) -> p c q s",
                                         c=NCH, q=TT, s=128)

            sel_tiles = {}

            def dma_xt(ci):
                tok = chunks[ci][1]
                xt_t = xtp.tile([128, EC, tok], F16, tag="xt", name="xt",
                                padded_shape=[128, EC, TOK])
                nc.sync.dma_start(xt_t[:], xt_r[:, ci, :, 0:tok])
                return xt_t

            def dma_selg(g):
                lo = g * SG
                gsz = min(SG, NCH - lo)
                selg = selp.tile([128, gsz, TT, 128], F16, tag="sel",
                                 name="sel", padded_shape=[128, SG, TT,
                                                           128])
                nc.gpsimd.dma_start(selg[:], sel_r[:, lo:lo + gsz, :, :])
                sel_tiles[g] = selg

            def l1(xt_t, tok):
                h1_t = actp.tile([128, HC, tok], F16, tag="h1", name="h1",
                                 bufs=4, padded_shape=[128, HC, TOK])
                for h in range(HC):
                    ps1 = psp.tile([128, tok], F32, tag="l1", name="l1",
                                   bufs=3, padded_shape=[128, TOK])
                    for e in range(EC):
                        _mm(nc, ps1[:], w1_t[:, h, e, :], xt_t[:, e, :],
                            start=(e == 0), stop=(e == EC - 1))
                    nc.scalar.activation(h1_t[:, h, :], ps1[:],
                                         mybir.ActivationFunctionType.Tanh,
                                         bias=b1_sb[:, h:h + 1])
                return h1_t

            def l2(h1_t, tok):
                tt = tok // 128
                h2_t = actp.tile([128, tt, H], F16, tag="h2", name="h2",
                                 padded_shape=[128, TT, H])
                for t in range(tt):
                    ps2 = psp.tile([128, H], F32, tag="l2", name="l2",
                                   bufs=3)
                    for k in range(HC):
                        _mm(nc, ps2[:], h1_t[:, k, t * 128:(t + 1) * 128],
                            w2_t[:, k, :], start=(k == 0),
                            stop=(k == HC - 1))
                    g2 = actp.tile([128, H], F16, tag="g2", name="g2")
                    nc.vector.tensor_add(g2[:], ps2[:], b2f_sb[:])
                    nc.scalar.activation(h2_t[:, t, :], g2[:],
                                         mybir.ActivationFunctionType.Tanh)
                return h2_t

            def seg(ci, h2_t, tok, is_first, is_last):
                tt = tok // 128
                selg = sel_tiles[ci // SG]
                for t in range(tt):
                    _mm(nc, enc_ps[:], selg[:, ci % SG, t, :],
                        h2_t[:, t, :],
                        start=(is_first and t == 0),
                        stop=(is_last and t == tt - 1))

            # ---- main loop: PE stream L1(i) | L2(i-2) | seg(i-3) ----
            assert NCH >= 4
            xt_q = [dma_xt(0), dma_xt(1), dma_xt(2)]
            dma_selg(0)
            h1_q = []
            h2_q = []
            for ci in range(NCH):
                if ci + 3 < NCH:
                    xt_q.append(dma_xt(ci + 3))
                if ci + 2 < NCH and (ci + 2) % SG == 0:
                    dma_selg((ci + 2) // SG)
                h1_q.append((l1(xt_q[ci], chunks[ci][1]), chunks[ci][1]))
                if ci >= 2:
                    h1_t, tok1 = h1_q[ci - 2]
                    h2_q.append((l2(h1_t, tok1), tok1))
                if ci >= 3:
                    h2_t, tok2 = h2_q[ci - 3]
                    seg(ci - 3, h2_t, tok2,
                        is_first=(ci == 3), is_last=False)
            # epilogue: both remaining L2 phases first, then the three
            # remaining seg phases (maximizes cover for the DVE+ACT chains)
            h2_q.append((l2(h1_q[NCH - 2][0], h1_q[NCH - 2][1]),
                         h1_q[NCH - 2][1]))
            h2_q.append((l2(h1_q[NCH - 1][0], h1_q[NCH - 1][1]),
                         h1_q[NCH - 1][1]))
            for sc in range(NCH - 3, NCH):
                seg(sc, h2_q[sc][0], h2_q[sc][1],
                    is_first=False, is_last=(sc == NCH - 1))

            # ---- predictor weights (gpsimd ring, behind the sel groups;
            # needed only at the very end) ----
            wp1_t = wp.tile([128, HC, HC, 128], F16, tag="wp1", name="wp1t")
            nc.gpsimd.dma_start(
                wp1_t[:], wp1_d.ap().rearrange("p (k h q) -> p k h q",
                                               k=HC, h=HC))
            p2_t = wp.tile([128, HC, HC, 128], F16, tag="p2", name="p2t")
            nc.gpsimd.dma_start(
                p2_t[:], p2_d.ap().rearrange("p (k h q) -> p k h q",
                                             k=HC, h=HC))
            p3_t = wp.tile([128, HC, O], F16, tag="p3", name="p3t")
            nc.gpsimd.dma_start(
                p3_t[:], p3_d.ap().rearrange("p (k o) -> p k o", k=HC))
            b3p1row = smp.tile([1, H], F32, tag="b3p1", name="b3p1")
            nc.gpsimd.dma_start(b3p1row[:], b3p1_d.ap())
            pb1_sb = smp.tile([128, HC], F32, tag="pb1", name="pb1")
            nc.gpsimd.dma_start(pb1_sb[:], pb1_d.ap())
            pb2_sb = smp.tile([128, HC], F32, tag="pb2", name="pb2")
            nc.gpsimd.dma_start(pb2_sb[:], pb2_d.ap())
            pb3row = smp.tile([1, O], F32, tag="pb3row", name="pb3row")
            nc.gpsimd.dma_start(pb3row[:], pb3_d.ap())
            cntrow = smp.tile([1, SLOTS], F32, tag="cntrow", name="cntrow")
            nc.gpsimd.dma_start(cntrow[:], cnt_d.ap())
            ones1 = smp.tile([1, SLOTS], F32, tag="ones1", name="ones1")
            nc.gpsimd.memset(ones1[:], 1.0)
            ident = smp.tile([SLOTS, SLOTS], F16, tag="ident", name="ident")
            make_identity(nc, ident[:])

            # ---- predictor on this core's own <=SLOTS segment rows ----
            # Z = segsum(h2) [SLOTS, H]; q1 = tanh(Z @ WP1 + cnt*b3p1 + pb1)
            # slice-pipelined: copy k-slice, transpose it while the next
            # slice copies, so the chain latency overlaps
            z_sb = smp.tile([SLOTS, H], F16, tag="zsb", name="zsb")
            zT = smp.tile([128, HC, SLOTS], F16, tag="zT", name="zT")
            psts = []
            nc.vector.tensor_copy(z_sb[:, 0:128], enc_ps[0:SLOTS, 0:128])
            for k in range(HC):
                if k + 1 < HC:
                    nc.vector.tensor_copy(
                        z_sb[:, (k + 1) * 128:(k + 2) * 128],
                        enc_ps[0:SLOTS, (k + 1) * 128:(k + 2) * 128])
                pst = psp.tile([128, SLOTS], F16, tag="l1", name="pst",
                               bufs=3)
                nc.tensor.transpose(pst[:], z_sb[:, k * 128:(k + 1) * 128],
                                    ident[:])
                nc.vector.tensor_copy(zT[:, k, :], pst[:])

            q1_sb = smp.tile([128, HC, SLOTS], F16, tag="q1", name="q1")
            for h in range(HC):
                pp1 = psp.tile([128, SLOTS], F32, tag="l1", name="pp1",
                               bufs=3)
                nc.tensor.matmul(pp1[:], b3p1row[:, h * 128:(h + 1) * 128],
                                 cntrow[:], start=True, stop=False,
                                 skip_group_check=True)
                for k in range(HC):
                    _mm(nc, pp1[:], wp1_t[:, k, h, :], zT[:, k, :],
                        start=False, stop=(k == HC - 1))
                nc.scalar.activation(q1_sb[:, h, :], pp1[:],
                                     mybir.ActivationFunctionType.Tanh,
                                     bias=pb1_sb[:, h:h + 1])
            q2_sb = smp.tile([128, HC, SLOTS], F16, tag="q2", name="q2")
            for h in range(HC):
                pp2 = psp.tile([128, SLOTS], F32, tag="l1", name="pp2",
                               bufs=3)
                for k in range(HC):
                    _mm(nc, pp2[:], p2_t[:, k, h, :], q1_sb[:, k, :],
                        start=(k == 0), stop=(k == HC - 1))
                nc.scalar.activation(q2_sb[:, h, :], pp2[:],
                                     mybir.ActivationFunctionType.Tanh,
                                     bias=pb2_sb[:, h:h + 1])

            # final: pred[slot, o] = q2.T @ P3 + pb3
            ppo = psp.tile([SLOTS, O], F32, tag="l2", name="ppo", bufs=3)
            nc.tensor.matmul(ppo[:], ones1[:], pb3row[:],
                             start=True, stop=False, skip_group_check=True)
            for k in range(HC):
                _mm(nc, ppo[:], q2_sb[:, k, :], p3_t[:, k, :],
                    start=False, stop=(k == HC - 1))
            pred_sb = smp.tile([SLOTS, O], F32, tag="pred", name="predsb")
            nc.vector.tensor_copy(pred_sb[:], ppo[:])
            nc.sync.dma_start(out_d.ap(), pred_sb[:])

    nc.compile()
    return nc


def kernel(words, seg_ids, W1, b1, W2, b2, W3, b3,
           P1, pb1, P2, pb2, P3, pb3, batch_size, alpha_iter, **_):
    words = np.asarray(words, dtype=np.float32)
    seg_ids = np.asarray(seg_ids).astype(np.int64)
    assert words.shape == (T, E), words.shape
    bs, ai = int(batch_size), int(alpha_iter)

    # --- host-side index prep: cut the sorted token axis at segment
    # boundaries so each core owns whole segments ---
    counts = np.bincount(seg_ids, minlength=S)[:S]
    starts = np.concatenate([[0], np.cumsum(counts)])   # [S+1]
    cuts = [0]
    for c in range(1, N_CORES):
        tgt = c * T // N_CORES
        j = int(np.searchsorted(starts, tgt, side="left"))
        if j > 0 and tgt - starts[j - 1] < starts[j] - tgt:
            j -= 1
        cuts.append(int(starts[j]))
    cuts.append(T)
    lens = np.diff(cuts)
    t_sh = int(np.ceil(lens.max() / 128) * 128)

    # contiguous segment range owned by each core
    seg_lo = [0] * N_CORES
    for c in range(N_CORES - 1, 0, -1):
        if lens[c] > 0:
            seg_lo[c] = int(seg_ids[cuts[c]])
        else:
            seg_lo[c] = S if c == N_CORES - 1 else seg_lo[c + 1]
    seg_hi = seg_lo[1:] + [S]
    slots_needed = max(seg_hi[c] - seg_lo[c] for c in range(N_CORES))
    SLOTS = min(128, max(MIN_SLOTS, ((slots_needed + 31) // 32) * 32))
    assert slots_needed <= SLOTS, (seg_lo, seg_hi)
    assert bs * ai == S

    xt = np.ascontiguousarray(words.T.astype(np.float16))    # [E, T] fp16
    n_full = t_sh // TOK
    tail = t_sh - n_full * TOK
    NCH = n_full + (1 if tail else 0)

    key = ("nc", t_sh, SLOTS)
    if key not in _CACHE:
        _CACHE[key] = _build_nc(t_sh, SLOTS)
    nc = _CACHE[key]

    # --- host-side weight pre-shuffles (dense [128, X] blocks) ---
    W1f, W2f = np.float32(W1), np.float32(W2)
    WP1 = (np.float32(W3) @ np.float32(P1)).astype(np.float32)  # [H, H]
    b3p1 = (np.float32(b3) @ np.float32(P1)).astype(np.float32)  # [H]

    def kmaj_tiles(Wm, dtype):
        # W [K, N] -> [128, K//128, N//128, 128] p-major
        K, N = Wm.shape
        return np.ascontiguousarray(
            Wm.reshape(K // 128, 128, N // 128, 128)
              .transpose(1, 0, 2, 3).reshape(128, -1).astype(dtype))

    common = {
        # w1 h-major: [128, HC, EC, 128]
        "w1": np.ascontiguousarray(
            W1f.astype(np.float16).reshape(EC, 128, HC, 128)
               .transpose(1, 2, 0, 3).reshape(128, -1)),
        "w2": np.ascontiguousarray(
            W2f.astype(np.float16).reshape(HC, 128, H)
               .transpose(1, 0, 2).reshape(128, -1)),
        "b1": np.ascontiguousarray(
            np.float32(b1).reshape(HC, 128).T),
        "b2f": np.ascontiguousarray(
            np.broadcast_to(np.float32(b2), (128, H))),
        "wp1": kmaj_tiles(WP1, np.float16),
        "b3p1": b3p1.reshape(1, H),
        "p2": kmaj_tiles(np.float32(P2), np.float16),
        "p3": np.ascontiguousarray(
            np.float16(P3).reshape(HC, 128, O)
              .transpose(1, 0, 2).reshape(128, -1)),
        "pb1": np.ascontiguousarray(np.float32(pb1).reshape(HC, 128).T),
        "pb2": np.ascontiguousarray(np.float32(pb2).reshape(HC, 128).T),
        "pb3": np.float32(pb3).reshape(1, O),
    }
    in_maps = []
    for c in range(N_CORES):
        lo, hi = cuts[c], cuts[c + 1]
        n = hi - lo
        # xt packed [128, NCH, EC, TOK]: 2KB lines per partition/chunk
        xt_flat = np.zeros((E, NCH * TOK), dtype=np.float16)
        xt_flat[:, :n] = xt[:, lo:hi]
        xt_c = np.ascontiguousarray(
            xt_flat.reshape(EC, 128, NCH, TOK)
                   .transpose(1, 2, 0, 3).reshape(128, -1))
        # packed one-hot selector: sel8[p, ci, q, s]
        sel_flat = np.zeros((NCH * TOK, SLOTS), dtype=np.float16)
        sel_flat[:n, :] = (seg_ids[lo:hi, None] ==
                           (seg_lo[c] + np.arange(SLOTS))[None, :])
        sel_pad = np.zeros((NCH * TOK, 128), dtype=np.float16)
        sel_pad[:, :SLOTS] = sel_flat
        sel_c = np.ascontiguousarray(
            sel_pad.reshape(NCH, TT, 128, 128)
                   .transpose(2, 0, 1, 3).reshape(128, -1))
        cnt_c = np.zeros((1, SLOTS), dtype=np.float32)
        nseg = seg_hi[c] - seg_lo[c]
        cnt_c[0, :nseg] = counts[seg_lo[c]:seg_hi[c]]
        in_maps.append({
            **common,
            "xt": xt_c,
            "sel": sel_c,
            "cnt": cnt_c,
        })

    global _LAST_IN_MAPS
    _LAST_IN_MAPS = in_maps
    res = bass_utils.run_bass_kernel_spmd(nc, in_maps,
                                          core_ids=list(range(N_CORES)))
    pred = np.zeros((S, O), dtype=np.float32)
    for c in range(N_CORES):
        nseg = seg_hi[c] - seg_lo[c]
        if nseg > 0:
            pred[seg_lo[c]:seg_hi[c]] = res.results[c]["pred"][:nseg]
    return pred.reshape(bs, ai, O).astype(np.float32)


_LAST_IN_MAPS = None


# revision 12
# speedup vs baseline: 1.0667x; 1.0029x over previous
"""DeepSetPred Trainium2 kernel: token encoder MLP + segment-sum + predictor
MLP on 8 NeuronCores, zero collectives.

Sharding: the host cuts the (sorted-by-segment) token axis at segment
boundaries, so every segment belongs to exactly one core. Each shard is
padded to a common length with tokens whose one-hot selector row is all
zero. Each core computes the complete segment sums for its own contiguous
range of <=SLOTS segments, runs the predictor on those rows, and writes its
private slice of the output; the host concatenates.

Structure: the encoder's third linear layer commutes with the segment sum
(it sits after the last tanh), so
    segsum(h2 @ W3 + b3) == segsum(h2) @ W3 + counts * b3
and W3 further folds into the predictor's first layer:
    enc @ P1 + pb1 == segsum(h2) @ (W3 @ P1) + counts * (b3 @ P1) + pb1.
The per-token path is only L1 + L2 + a one-hot segsum matmul over h2
(14336 PE rows per 512-token chunk). L2 is computed token-major (h1 tile
stationary, W2 moving) so the segsum needs no transpose; its bias is added
by the DVE from a broadcast tile (ACT bias is per-partition only), then ACT
applies tanh. The PE stream is skewed L1(i) | L2(i-2) | seg(i-3) so neither
the w2 weight DMA at startup nor the DVE+ACT hop ever stalls the PE. All
weights are host-pre-shuffled into dense [128, X] partition-contiguous
blocks (xt uses 2KB partition lines on the sync queue; w1/b1 ride the
scalar queue, w2/b2/sel the gpsimd queue, predictor weights load behind
the sel stream), and sel is padded to 128 slots so the seg matmul keeps
the full 128-column PE configuration. The predictor runs in fp16.
"""

import numpy as np

import concourse.mybir as mybir
import concourse.tile as tile
from concourse import bacc
from concourse import bass_utils
from concourse.masks import make_identity

# Problem shapes (hardcoded per contract).
T, E, H, C, O = 131072, 256, 512, 256, 32
S = 128            # num segments
N_CORES = 8
TOK = 512          # tokens per chunk
MIN_SLOTS = 32     # baseline segments-per-core capacity
SG = 4             # chunks per sel DMA group
F32 = mybir.dt.float32
F32R = mybir.dt.float32r
F16 = mybir.dt.float16

EC = E // 128   # 2
HC = H // 128   # 4
TT = TOK // 128  # 4 token sub-tiles per chunk

_CACHE = {}


def _mm(nc, out, lhsT, rhs, start, stop, skip=True):
    nc.tensor.matmul(out, lhsT, rhs,
                     start=start, stop=stop, skip_group_check=skip)


def _build_nc(t_sh, SLOTS):
    assert t_sh % 128 == 0
    n_full = t_sh // TOK
    tail = t_sh - n_full * TOK
    chunks = [(i * TOK, TOK) for i in range(n_full)]
    if tail:
        chunks.append((n_full * TOK, tail))
    NCH = len(chunks)
    NSG = (NCH + SG - 1) // SG

    nc = bacc.Bacc("TRN2", target_bir_lowering=False, debug=False,
                   num_devices=N_CORES)

    # xt packed: [128, NCH, EC, TOK] -> 2KB contiguous per partition/chunk
    xt_d = nc.dram_tensor("xt", [128, NCH * EC * TOK], F16,
                          kind="ExternalInput")
    # sel packed per chunk, slot dim padded to 128 so the seg matmul keeps
    # the full 128-column stationary config (no PE col_grp switch)
    sel_d = nc.dram_tensor("sel", [128, NCH * TT * 128], F16,
                           kind="ExternalInput")
    cnt_d = nc.dram_tensor("cnt", [1, SLOTS], F32, kind="ExternalInput")
    # dense pre-shuffled weights: [128, ...] partition-major blocks
    w1_d = nc.dram_tensor("w1", [128, HC * EC * 128], F16,
                          kind="ExternalInput")      # h-major tiles
    w2_d = nc.dram_tensor("w2", [128, HC * H], F16, kind="ExternalInput")
    b1_d = nc.dram_tensor("b1", [128, HC], F32, kind="ExternalInput")
    b2f_d = nc.dram_tensor("b2f", [128, H], F32, kind="ExternalInput")
    wp1_d = nc.dram_tensor("wp1", [128, HC * HC * 128], F16,
                           kind="ExternalInput")   # W3 @ P1, k-major tiles
    b3p1_d = nc.dram_tensor("b3p1", [1, H], F32, kind="ExternalInput")
    p2_d = nc.dram_tensor("p2", [128, HC * HC * 128], F16,
                          kind="ExternalInput")
    p3_d = nc.dram_tensor("p3", [128, HC * O], F16, kind="ExternalInput")
    pb1_d = nc.dram_tensor("pb1", [128, HC], F32, kind="ExternalInput")
    pb2_d = nc.dram_tensor("pb2", [128, HC], F32, kind="ExternalInput")
    pb3_d = nc.dram_tensor("pb3", [1, O], F32, kind="ExternalInput")
    out_d = nc.dram_tensor("pred", [SLOTS, O], F32, kind="ExternalOutput")

    with tile.TileContext(nc) as tc:
        with tc.tile_pool(name="wts", bufs=1) as wp, \
             tc.tile_pool(name="xt", bufs=5) as xtp, \
             tc.tile_pool(name="sel", bufs=3) as selp, \
             tc.tile_pool(name="act", bufs=3) as actp, \
             tc.tile_pool(name="small", bufs=1) as smp, \
             tc.tile_pool(name="ps", bufs=2, space="PSUM") as psp, \
             tc.tile_pool(name="psacc", bufs=1, space="PSUM") as psa:

            # warm the ACT tanh table before the queues fill
            warm_sb = smp.tile([1, 1], F32, tag="warm", name="warm")
            nc.gpsimd.memset(warm_sb[:], 0.0)
            warm_o = smp.tile([1, 1], F32, tag="warmo", name="warmo")
            nc.scalar.activation(warm_o[:], warm_sb[:],
                                 mybir.ActivationFunctionType.Tanh)

            # ---- resident weights; every DMA is partition-contiguous.
            # w1/w2 split across the scalar+vector queues so both halves
            # land in parallel while the sync queue streams xt. ----
            # w1 (whole, 2KB lines) + b1 on the scalar ring; w2 (whole,
            # 4KB lines) leads the gpsimd ring so L2(0) is never blocked.
            w1_t = wp.tile([128, HC, EC, 128], F16, tag="w1", name="w1t")
            w1_r = w1_d.ap().rearrange("p (h e q) -> p h e q", h=HC, e=EC)
            nc.scalar.dma_start(w1_t[:], w1_r)
            b1_sb = smp.tile([128, HC], F32, tag="b1", name="b1")
            nc.scalar.dma_start(b1_sb[:], b1_d.ap())
            w2_t = wp.tile([128, HC, H], F16, tag="w2", name="w2t")
            w2_r = w2_d.ap().rearrange("p (k j) -> p k j", k=HC)
            nc.gpsimd.dma_start(w2_t[:], w2_r)
            b2f_sb = smp.tile([128, H], F32, tag="b2f", name="b2f")
            nc.gpsimd.dma_start(b2f_sb[:], b2f_d.ap())

            # ---- persistent segment-sum accumulator Z[slot, h] ----
            enc_ps = psa.tile([128, H], F32, tag="encacc", name="encacc")

            xt_r = xt_d.ap().rearrange("p (c e t) -> p c e t", c=NCH, e=EC)
            sel_r = sel_d.ap().rearrange("p (c q s) -> p c q s",
                                         c=NCH, q=TT, s=128)

            sel_tiles = {}

            def dma_xt(ci):
                tok = chunks[ci][1]
                xt_t = xtp.tile([128, EC, tok], F16, tag="xt", name="xt",
                                padded_shape=[128, EC, TOK])
                nc.sync.dma_start(xt_t[:], xt_r[:, ci, :, 0:tok])
                return xt_t

            def dma_selg(g):
                lo = g * SG
                gsz = min(SG, NCH - lo)
                selg = selp.tile([128, gsz, TT, 128], F16, tag="sel",
                                 name="sel", padded_shape=[128, SG, TT,
                                                           128])
                nc.gpsimd.dma_start(selg[:], sel_r[:, lo:lo + gsz, :, :])
                sel_tiles[g] = selg

            def l1(xt_t, tok):
                h1_t = actp.tile([128, HC, tok], F16, tag="h1", name="h1",
                                 bufs=4, padded_shape=[128, HC, TOK])
                for h in range(HC):
                    ps1 = psp.tile([128, tok], F32, tag="l1", name="l1",
                                   bufs=3, padded_shape=[128, TOK])
                    for e in range(EC):
                        _mm(nc, ps1[:], w1_t[:, h, e, :], xt_t[:, e, :],
                            start=(e == 0), stop=(e == EC - 1))
                    nc.scalar.activation(h1_t[:, h, :], ps1[:],
                                         mybir.ActivationFunctionType.Tanh,
                                         bias=b1_sb[:, h:h + 1])
                return h1_t

            def l2(h1_t, tok):
                tt = tok // 128
                h2_t = actp.tile([128, tt, H], F16, tag="h2", name="h2",
                                 padded_shape=[128, TT, H])
                for t in range(tt):
                    ps2 = psp.tile([128, H], F32, tag="l2", name="l2",
                                   bufs=3)
                    for k in range(HC):
                        _mm(nc, ps2[:], h1_t[:, k, t * 128:(t + 1) * 128],
                            w2_t[:, k, :], start=(k == 0),
                            stop=(k == HC - 1))
                    g2 = actp.tile([128, H], F16, tag="g2", name="g2")
                    nc.vector.tensor_add(g2[:], ps2[:], b2f_sb[:])
                    nc.scalar.activation(h2_t[:, t, :], g2[:],
                                         mybir.ActivationFunctionType.Tanh)
                return h2_t

            def seg(ci, h2_t, tok, is_first, is_last):
                tt = tok // 128
                selg = sel_tiles[ci // SG]
                for t in range(tt):
                    _mm(nc, enc_ps[:], selg[:, ci % SG, t, :],
                        h2_t[:, t, :],
                        start=(is_first and t == 0),
                        stop=(is_last and t == tt - 1))

            # ---- main loop: PE stream L1(i) | L2(i-2) | seg(i-3) ----
            assert NCH >= 4
            xt_q = [dma_xt(0), dma_xt(1), dma_xt(2)]
            dma_selg(0)
            h1_q = []
            h2_q = []
            for ci in range(NCH):
                if ci + 3 < NCH:
                    xt_q.append(dma_xt(ci + 3))
                if ci + 2 < NCH and (ci + 2) % SG == 0:
                    dma_selg((ci + 2) // SG)
                h1_q.append((l1(xt_q[ci], chunks[ci][1]), chunks[ci][1]))
                if ci >= 2:
                    h1_t, tok1 = h1_q[ci - 2]
                    h2_q.append((l2(h1_t, tok1), tok1))
                if ci >= 3:
                    h2_t, tok2 = h2_q[ci - 3]
                    seg(ci - 3, h2_t, tok2,
                        is_first=(ci == 3), is_last=False)
            # epilogue: both remaining L2 phases first, then the three
            # remaining seg phases (maximizes cover for the DVE+ACT chains)
            h2_q.append((l2(h1_q[NCH - 2][0], h1_q[NCH - 2][1]),
                         h1_q[NCH - 2][1]))
            h2_q.append((l2(h1_q[NCH - 1][0], h1_q[NCH - 1][1]),
                         h1_q[NCH - 1][1]))
            for sc in range(NCH - 3, NCH):
                seg(sc, h2_q[sc][0], h2_q[sc][1],
                    is_first=False, is_last=(sc == NCH - 1))

            # ---- predictor weights (gpsimd ring, behind the sel groups;
            # needed only at the very end) ----
            wp1_t = wp.tile([128, HC, HC, 128], F16, tag="wp1", name="wp1t")
            nc.gpsimd.dma_start(
                wp1_t[:], wp1_d.ap().rearrange("p (k h q) -> p k h q",
                                               k=HC, h=HC))
            p2_t = wp.tile([128, HC, HC, 128], F16, tag="p2", name="p2t")
            nc.gpsimd.dma_start(
                p2_t[:], p2_d.ap().rearrange("p (k h q) -> p k h q",
                                             k=HC, h=HC))
            p3_t = wp.tile([128, HC, O], F16, tag="p3", name="p3t")
            nc.gpsimd.dma_start(
                p3_t[:], p3_d.ap().rearrange("p (k o) -> p k o", k=HC))
            b3p1row = smp.tile([1, H], F32, tag="b3p1", name="b3p1")
            nc.gpsimd.dma_start(b3p1row[:], b3p1_d.ap())
            pb1_sb = smp.tile([128, HC], F32, tag="pb1", name="pb1")
            nc.gpsimd.dma_start(pb1_sb[:], pb1_d.ap())
            pb2_sb = smp.tile([128, HC], F32, tag="pb2", name="pb2")
            nc.gpsimd.dma_start(pb2_sb[:], pb2_d.ap())
            pb3row = smp.tile([1, O], F32, tag="pb3row", name="pb3row")
            nc.gpsimd.dma_start(pb3row[:], pb3_d.ap())
            cntrow = smp.tile([1, SLOTS], F32, tag="cntrow", name="cntrow")
            nc.gpsimd.dma_start(cntrow[:], cnt_d.ap())
            ones1 = smp.tile([1, SLOTS], F32, tag="ones1", name="ones1")
            nc.gpsimd.memset(ones1[:], 1.0)
            ident = smp.tile([SLOTS, SLOTS], F16, tag="ident", name="ident")
            make_identity(nc, ident[:])

            # ---- predictor on this core's own <=SLOTS segment rows ----
            # Z = segsum(h2) [SLOTS, H]; q1 = tanh(Z @ WP1 + cnt*b3p1 + pb1)
            # slice-pipelined: copy k-slice, transpose it while the next
            # slice copies, so the chain latency overlaps
            z_sb = smp.tile([SLOTS, H], F16, tag="zsb", name="zsb")
            zT = smp.tile([128, HC, SLOTS], F16, tag="zT", name="zT")
            psts = []
            nc.vector.tensor_copy(z_sb[:, 0:128], enc_ps[0:SLOTS, 0:128])
            for k in range(HC):
                if k + 1 < HC:
                    nc.vector.tensor_copy(
                        z_sb[:, (k + 1) * 128:(k + 2) * 128],
                        enc_ps[0:SLOTS, (k + 1) * 128:(k + 2) * 128])
                pst = psp.tile([128, SLOTS], F16, tag="l1", name="pst",
                               bufs=3)
                nc.tensor.transpose(pst[:], z_sb[:, k * 128:(k + 1) * 128],
                                    ident[:])
                nc.vector.tensor_copy(zT[:, k, :], pst[:])

            q1_sb = smp.tile([128, HC, SLOTS], F16, tag="q1", name="q1")
            for h in range(HC):
                pp1 = psp.tile([128, SLOTS], F32, tag="l1", name="pp1",
                               bufs=3)
                nc.tensor.matmul(pp1[:], b3p1row[:, h * 128:(h + 1) * 128],
                                 cntrow[:], start=True, stop=False,
                                 skip_group_check=True)
                for k in range(HC):
                    _mm(nc, pp1[:], wp1_t[:, k, h, :], zT[:, k, :],
                        start=False, stop=(k == HC - 1))
                nc.scalar.activation(q1_sb[:, h, :], pp1[:],
                                     mybir.ActivationFunctionType.Tanh,
                                     bias=pb1_sb[:, h:h + 1])
            q2_sb = smp.tile([128, HC, SLOTS], F16, tag="q2", name="q2")
            for h in range(HC):
                pp2 = psp.tile([128, SLOTS], F32, tag="l1", name="pp2",
                               bufs=3)
                for k in range(HC):
                    _mm(nc, pp2[:], p2_t[:, k, h, :], q1_sb[:, k, :],
                        start=(k == 0), stop=(k == HC - 1))
                nc.scalar.activation(q2_sb[:, h, :], pp2[:],
                                     mybir.ActivationFunctionType.Tanh,
                                     bias=pb2_sb[:, h:h + 1])

            # final: pred[slot, o] = q2.T @ P3 + pb3
            ppo = psp.tile([SLOTS, O], F32, tag="l2", name="ppo", bufs=3)
            nc.tensor.matmul(ppo[:], ones1[:], pb3row[:],
                             start=True, stop=False, skip_group_check=True)
            for k in range(HC):
                _mm(nc, ppo[:], q2_sb[:, k, :], p3_t[:, k, :],
                    start=False, stop=(k == HC - 1))
            pred_sb = smp.tile([SLOTS, O], F32, tag="pred", name="predsb")
            nc.vector.tensor_copy(pred_sb[:], ppo[:])
            nc.sync.dma_start(out_d.ap(), pred_sb[:])

    nc.compile()
    return nc


def kernel(words, seg_ids, W1, b1, W2, b2, W3, b3,
           P1, pb1, P2, pb2, P3, pb3, batch_size, alpha_iter, **_):
    words = np.asarray(words, dtype=np.float32)
    seg_ids = np.asarray(seg_ids).astype(np.int64)
    assert words.shape == (T, E), words.shape
    bs, ai = int(batch_size), int(alpha_iter)

    # --- host-side index prep: cut the sorted token axis at segment
    # boundaries so each core owns whole segments ---
    counts = np.bincount(seg_ids, minlength=S)[:S]
    starts = np.concatenate([[0], np.cumsum(counts)])   # [S+1]
    cuts = [0]
    for c in range(1, N_CORES):
        tgt = c * T // N_CORES
        j = int(np.searchsorted(starts, tgt, side="left"))
        if j > 0 and tgt - starts[j - 1] < starts[j] - tgt:
            j -= 1
        cuts.append(int(starts[j]))
    cuts.append(T)
    lens = np.diff(cuts)
    t_sh = int(np.ceil(lens.max() / 128) * 128)

    # contiguous segment range owned by each core
    seg_lo = [0] * N_CORES
    for c in range(N_CORES - 1, 0, -1):
        if lens[c] > 0:
            seg_lo[c] = int(seg_ids[cuts[c]])
        else:
            seg_lo[c] = S if c == N_CORES - 1 else seg_lo[c + 1]
    seg_hi = seg_lo[1:] + [S]
    slots_needed = max(seg_hi[c] - seg_lo[c] for c in range(N_CORES))
    SLOTS = min(128, max(MIN_SLOTS, ((slots_needed + 31) // 32) * 32))
    assert slots_needed <= SLOTS, (seg_lo, seg_hi)
    assert bs * ai == S

    xt = np.ascontiguousarray(words.T.astype(np.float16))    # [E, T] fp16
    n_full = t_sh // TOK
    tail = t_sh - n_full * TOK
    NCH = n_full + (1 if tail else 0)

    key = ("nc", t_sh, SLOTS)
    if key not in _CACHE:
        _CACHE[key] = _build_nc(t_sh, SLOTS)
    nc = _CACHE[key]

    # --- host-side weight pre-shuffles (dense [128, X] blocks) ---
    W1f, W2f = np.float32(W1), np.float32(W2)
    WP1 = (np.float32(W3) @ np.float32(P1)).astype(np.float32)  # [H, H]
    b3p1 = (np.float32(b3) @ np.float32(P1)).astype(np.float32)  # [H]

    def kmaj_tiles(Wm, dtype):
        # W [K, N] -> [128, K//128, N//128, 128] p-major
        K, N = Wm.shape
        return np.ascontiguousarray(
            Wm.reshape(K // 128, 128, N // 128, 128)
              .transpose(1, 0, 2, 3).reshape(128, -1).astype(dtype))

    common = {
        # w1 h-major: [128, HC, EC, 128]
        "w1": np.ascontiguousarray(
            W1f.astype(np.float16).reshape(EC, 128, HC, 128)
               .transpose(1, 2, 0, 3).reshape(128, -1)),
        "w2": np.ascontiguousarray(
            W2f.astype(np.float16).reshape(HC, 128, H)
               .transpose(1, 0, 2).reshape(128, -1)),
        "b1": np.ascontiguousarray(
            np.float32(b1).reshape(HC, 128).T),
        "b2f": np.ascontiguousarray(
            np.broadcast_to(np.float32(b2), (128, H))),
        "wp1": kmaj_tiles(WP1, np.float16),
        "b3p1": b3p1.reshape(1, H),
        "p2": kmaj_tiles(np.float32(P2), np.float16),
        "p3": np.ascontiguousarray(
            np.float16(P3).reshape(HC, 128, O)
              .transpose(1, 0, 2).reshape(128, -1)),
        "pb1": np.ascontiguousarray(np.float32(pb1).reshape(HC, 128).T),
        "pb2": np.ascontiguousarray(np.float32(pb2).reshape(HC, 128).T),
        "pb3": np.float32(pb3).reshape(1, O),
    }
    in_maps = []
    for c in range(N_CORES):
        lo, hi = cuts[c], cuts[c + 1]
        n = hi - lo
        # xt packed [128, NCH, EC, TOK]: 2KB lines per partition/chunk
        xt_flat = np.zeros((E, NCH * TOK), dtype=np.float16)
        xt_flat[:, :n] = xt[:, lo:hi]
        xt_c = np.ascontiguousarray(
            xt_flat.reshape(EC, 128, NCH, TOK)
                   .transpose(1, 2, 0, 3).reshape(128, -1))
        # packed one-hot selector: sel8[p, ci, q, s]
        sel_flat = np.zeros((NCH * TOK, SLOTS), dtype=np.float16)
        sel_flat[:n, :] = (seg_ids[lo:hi, None] ==
                           (seg_lo[c] + np.arange(SLOTS))[None, :])
        sel_pad = np.zeros((NCH * TOK, 128), dtype=np.float16)
        sel_pad[:, :SLOTS] = sel_flat
        sel_c = np.ascontiguousarray(
            sel_pad.reshape(NCH, TT, 128, 128)
                   .transpose(2, 0, 1, 3).reshape(128, -1))
        cnt_c = np.zeros((1, SLOTS), dtype=np.float32)
        nseg = seg_hi[c] - seg_lo[c]
        cnt_c[0, :nseg] = counts[seg_lo[c]:seg_hi[c]]
        in_maps.append({
            **common,
            "xt": xt_c,
            "sel": sel_c,
            "cnt": cnt_c,
        })

    global _LAST_IN_MAPS
    _LAST_IN_MAPS = in_maps
    res = bass_utils.run_bass_kernel_spmd(nc, in_maps,
                                          core_ids=list(range(N_CORES)))
    pred = np.zeros((S, O), dtype=np.float32)
    for c in range(N_CORES):
        nseg = seg_hi[c] - seg_lo[c]
        if nseg > 0:
            pred[seg_lo[c]:seg_hi[c]] = res.results[c]["pred"][:nseg]
    return pred.reshape(bs, ai, O).astype(np.float32)


_LAST_IN_MAPS = None
